# revision 20
# baseline (speedup 1.0000x reference)
"""AttentionBlock (GroupNorm + QKV + 8-head spatial attention + proj + residual)
on 8 Trainium2 NeuronCores.

Sharding: 16 head-batches (B=2 x NH=8) are split 2-per-core; cores 0-3 take
batch 0, cores 4-7 batch 1.  Each core:
  - loads its batch's x [512, 4096] and computes GroupNorm statistics on-chip
    (bn_stats per channel, group-combine + group->channel broadcast via tiny
    mask matmuls on the PE),
  - folds the GroupNorm affine into the QKV weights (W' = W*A per channel,
    bias' = W@B + qkv_b) so x feeds the QKV matmuls directly,
  - computes q/k for its 2 heads in [c, L] layout and v TRANSPOSED ([L, c])
    straight out of the QKV matmul (x^T @ Wv'^T) so attention needs no
    on-chip transposes,
  - scores are computed in [s, t] layout; softmax denominators come free from
    an extra ones-column in vT (a_plus row 64); exp is done without
    max-subtraction (scores are ~N(0,1) here, exact softmax identity),
  - emits its partial projection  proj_w[:, head_cols] @ a  [512, 4096].
Host sums the 4 partials per batch, adds proj_b and the residual.

All matmuls run as float32r (TF32-like: full PE rate, ~1e-3 worst-case
relative error vs fp32 measured on HW).
"""

import numpy as np

import concourse.bacc as bacc
import concourse.tile as tile
from concourse import mybir
from concourse.bass_utils import run_bass_kernel_spmd

B, C = 2, 512
L = 64 * 64           # 4096
NH = 8                # heads total
CH = 64               # channels per head
G = 32                # groups
EPS = 1e-5
N_CORES = 8
HEADS_PER_CORE = 2

F32 = mybir.dt.float32
F32R = mybir.dt.float32r
AF = mybir.ActivationFunctionType
ALU = mybir.AluOpType

TSUP = 1024           # t-stripe width (2 PSUM banks)
NT = L // TSUP        # 2 stripes
SJ = 32               # number of 128-wide s-chunks


def _f(ap):
    return ap.bitcast(F32)


_PROGRAM = None


def build_program():
    nc = bacc.Bacc()
    xb = nc.declare_dram_parameter("xb", [C, L], F32R, isOutput=False).ap()
    gmask = nc.declare_dram_parameter("gmask", [128, 4, G], F32R, isOutput=False).ap()
    bmask = nc.declare_dram_parameter("bmask", [G, 4, 128], F32R, isOutput=False).ap()
    gamma4 = nc.declare_dram_parameter("gamma4", [4, 128], F32, isOutput=False).ap()
    beta4 = nc.declare_dram_parameter("beta4", [4, 128], F32, isOutput=False).ap()
    wqT = nc.declare_dram_parameter("wqT", [C, 128], F32R, isOutput=False).ap()
    wkT = nc.declare_dram_parameter("wkT", [C, 128], F32R, isOutput=False).ap()
    wvT = nc.declare_dram_parameter("wvT", [C, 256], F32R, isOutput=False).ap()
    qb = nc.declare_dram_parameter("qb", [128], F32, isOutput=False).ap()
    kb = nc.declare_dram_parameter("kb", [128], F32, isOutput=False).ap()
    vb = nc.declare_dram_parameter("vb", [130], F32, isOutput=False).ap()
    pwT = nc.declare_dram_parameter("pwT", [128, C], F32R, isOutput=False).ap()
    part = nc.declare_dram_parameter("part", [C, L], F32, isOutput=True).ap()

    with tile.TileContext(nc) as tc:
        with (
            tc.tile_pool(name="consts", bufs=1) as consts,
            tc.tile_pool(name="big", bufs=1) as big,
            tc.tile_pool(name="work", bufs=2) as work,
            tc.tile_pool(name="ps", bufs=1, space="PSUM") as ps,
        ):
            # ---- constants into SBUF ----
            sb_gmask = consts.tile([128, 4, G], F32R)
            nc.sync.dma_start(out=sb_gmask, in_=gmask)
            sb_bmask = consts.tile([G, 4, 128], F32R)
            nc.sync.dma_start(out=sb_bmask, in_=bmask)
            sb_gamma = consts.tile([128, 4], F32)
            nc.sync.dma_start(out=sb_gamma, in_=gamma4.rearrange("t p -> p t"))
            sb_beta = consts.tile([128, 4], F32)
            nc.sync.dma_start(out=sb_beta, in_=beta4.rearrange("t p -> p t"))
            sb_wq = consts.tile([128, 4, 128], F32R)
            nc.sync.dma_start(out=sb_wq, in_=wqT.rearrange("(kk p) m -> p kk m", p=128))
            sb_wk = consts.tile([128, 4, 128], F32R)
            nc.sync.dma_start(out=sb_wk, in_=wkT.rearrange("(kk p) m -> p kk m", p=128))
            sb_wv = consts.tile([128, 4, 256], F32R)
            nc.sync.dma_start(out=sb_wv, in_=wvT.rearrange("(kk p) m -> p kk m", p=128))
            sb_pw = consts.tile([128, C], F32R)
            nc.sync.dma_start(out=sb_pw, in_=pwT)
            sb_qb = consts.tile([128, 1], F32)
            nc.sync.dma_start(out=sb_qb, in_=qb.unsqueeze(1))
            sb_kb = consts.tile([128, 1], F32)
            nc.sync.dma_start(out=sb_kb, in_=kb.unsqueeze(1))
            sb_vb = consts.tile([1, 130], F32)
            nc.sync.dma_start(out=sb_vb, in_=vb.unsqueeze(0))
            eps32 = consts.tile([32, 1], F32)
            nc.vector.memset(eps32, EPS)
            mh0 = consts.tile([128, 1], F32)
            nc.vector.memset(mh0[0:64, :], 1.0)
            nc.vector.memset(mh0[64:128, :], 0.0)
            mh1 = consts.tile([128, 1], F32)
            nc.vector.memset(mh1[0:64, :], 0.0)
            nc.vector.memset(mh1[64:128, :], 1.0)

            # ---- load x ----
            xt = big.tile([128, 4, L], F32R)
            xbr = xb.rearrange("(t p) l -> p t l", p=128)
            stats = work.tile([128, 4, 8, 6], F32, bufs=1)
            for s in range(8):
                ns = slice(s * 512, (s + 1) * 512)
                nc.sync.dma_start(out=xt[:, :, ns], in_=xbr[:, :, ns])
                for t in range(4):
                    nc.vector.bn_stats(
                        out=stats[:, t, s, :], in_=_f(xt[:, t, ns])
                    )
            mv = work.tile([128, 4, 2], F32, bufs=1)
            for t in range(4):
                nc.vector.bn_aggr(out=mv[:, t, :], in_=stats[:, t, :, :])
            # per-channel [mean, var+mean^2]
            stats2 = work.tile([128, 4, 2], F32R, bufs=1)
            msq = work.tile([128, 4, 1], F32, bufs=1)
            nc.vector.tensor_copy(out=stats2[:, :, 0:1], in_=mv[:, :, 0:1])
            nc.vector.tensor_mul(msq, mv[:, :, 0:1], mv[:, :, 0:1])
            nc.vector.tensor_add(stats2[:, :, 1:2], mv[:, :, 1:2], msq)
            # group stats via mask matmul: [32, 2] = (mean_g, E[x^2]_g)
            gps = ps.tile([32, 2], F32, tag="apl0")
            for t in range(4):
                nc.tensor.matmul(
                    gps, sb_gmask[:, t, :], stats2[:, t, :],
                    start=(t == 0), stop=(t == 3),
                )
            gs = work.tile([32, 2], F32, bufs=1)
            nc.vector.tensor_copy(out=gs, in_=gps)
            msqg = work.tile([32, 1], F32, bufs=1)
            varg = work.tile([32, 1], F32, bufs=1)
            nc.vector.tensor_mul(msqg, gs[:, 0:1], gs[:, 0:1])
            nc.vector.tensor_sub(varg, gs[:, 1:2], msqg)
            # rstd = exp(-0.5*ln(var+eps))  (Ln+Exp share one ACT table set)
            lng = work.tile([32, 1], F32, bufs=1)
            nc.scalar.activation(out=lng, in_=varg, func=AF.Ln, bias=eps32, scale=1.0)
            rstdg = work.tile([32, 1], F32, bufs=1)
            nc.scalar.activation(out=rstdg, in_=lng, func=AF.Exp, scale=-0.5)
            gstats2 = work.tile([32, 2], F32R, bufs=1)
            nc.vector.tensor_copy(out=gstats2[:, 0:1], in_=gs[:, 0:1])
            nc.vector.tensor_copy(out=gstats2[:, 1:2], in_=rstdg)

            # ---- per-channel affine A, Bs  (hid = x*A + Bs) ----
            A_all = work.tile([128, 4], F32, bufs=1)
            Bcol = work.tile([128, 4, 2], F32R, bufs=1)
            for t in range(4):
                cst = ps.tile([128, 2], F32, tag="apl1")
                nc.tensor.matmul(
                    cst, sb_bmask[:, t, :], gstats2, start=True, stop=True
                )
                nc.vector.tensor_mul(A_all[:, t:t + 1], cst[:, 1:2], sb_gamma[:, t:t + 1])
                tmp = work.tile([128, 1], F32, tag="tmp")
                nc.vector.tensor_mul(tmp, cst[:, 0:1], A_all[:, t:t + 1])
                nc.vector.tensor_sub(Bcol[:, t, :], sb_beta[:, t:t + 1].broadcast_to([128, 2]), tmp.broadcast_to([128, 2]))

            # ---- fold affine into QKV weights ----
            # bias' = W^T @ Bs + b first (reads original W), then W *= A in place
            cq_ps = ps.tile([128, 2], F32, tag="sc", bufs=2)
            ck_ps = ps.tile([128, 2], F32, tag="apl2")
            cv_ps = ps.tile([1, 256], F32, tag="apl0")
            for t in range(4):
                nc.tensor.matmul(cq_ps, sb_wq[:, t, :], Bcol[:, t, :],
                                 start=(t == 0), stop=(t == 3))
                nc.tensor.matmul(ck_ps, sb_wk[:, t, :], Bcol[:, t, :],
                                 start=(t == 0), stop=(t == 3))
                nc.tensor.matmul(cv_ps, Bcol[:, t, 0:1], sb_wv[:, t, :],
                                 start=(t == 0), stop=(t == 3))
            qc = consts.tile([128, 1], F32)
            nc.vector.tensor_add(qc, cq_ps[:, 0:1], sb_qb)
            kc = consts.tile([128, 1], F32)
            nc.vector.tensor_add(kc, ck_ps[:, 0:1], sb_kb)
            vrow = work.tile([1, 130], F32, bufs=1)
            nc.vector.tensor_add(vrow, cv_ps[:, 0:130], sb_vb)
            vbc = consts.tile([128, 130], F32)
            nc.gpsimd.partition_broadcast(vbc, vrow)
            for t in range(4):
                nc.vector.tensor_scalar_mul(
                    out=sb_wq[:, t, :], in0=_f(sb_wq[:, t, :]), scalar1=A_all[:, t:t + 1])
                nc.vector.tensor_scalar_mul(
                    out=sb_wk[:, t, :], in0=_f(sb_wk[:, t, :]), scalar1=A_all[:, t:t + 1])
                nc.vector.tensor_scalar_mul(
                    out=sb_wv[:, t, :], in0=_f(sb_wv[:, t, :]), scalar1=A_all[:, t:t + 1])

            # ---- QKV ----
            q2 = big.tile([128, L], F32R)
            k2z = [big.tile([128, L], F32R, name="k2z0"),
                   big.tile([128, L], F32R, name="k2z1")]
            for n in range(8):
                ns = slice(n * 512, (n + 1) * 512)
                qp = ps.tile([128, 512], F32, tag="sc", bufs=2, name="qp")
                for kk in range(4):
                    nc.tensor.matmul(qp, sb_wq[:, kk, :], xt[:, kk, ns],
                                     start=(kk == 0), stop=(kk == 3))
                nc.vector.tensor_scalar_add(out=q2[:, ns], in0=qp, scalar1=qc)
                kp = ps.tile([128, 512], F32, tag=("apl2" if n % 2 == 0 else "apl3"), name="kp")
                for kk in range(4):
                    nc.tensor.matmul(kp, sb_wk[:, kk, :], xt[:, kk, ns],
                                     start=(kk == 0), stop=(kk == 3))
                # (k + kc) masked per head: other head's partitions zeroed so the
                # scores matmul can contract over all 128 partitions (K=128 is
                # 2x faster than K=64 for f32r)
                nc.vector.tensor_scalar(out=k2z[0][:, ns], in0=kp, scalar1=kc,
                                        scalar2=mh0, op0=ALU.add, op1=ALU.mult)
                nc.vector.tensor_scalar(out=k2z[1][:, ns], in0=kp, scalar1=kc,
                                        scalar2=mh1, op0=ALU.add, op1=ALU.mult)
            # vT: [s, c] both heads + ones cols at 64 (h0) / 129 (h1)
            vt = big.tile([128, SJ, 130], F32R)
            for j in range(SJ):
                js = slice(j * 128, (j + 1) * 128)
                vp = ps.tile([128, 256], F32, tag=("apl0" if j % 2 == 0 else "apl1"), name="vp")
                for kk in range(4):
                    nc.tensor.matmul(vp, xt[:, kk, js], sb_wv[:, kk, :],
                                     start=(kk == 0), stop=(kk == 3))
                nc.vector.tensor_add(vt[:, j, 0:130], vp[:, 0:130], vbc)

            # ---- attention ----
            # Both heads interleaved per j: two independent scores->exp chains
            # keep ACT (the bottleneck) saturated while the PE stays warm.
            a_cat = big.tile([128, L], F32R, tag="xt")

            def emit_normalize(ts_idx, acp_t):
                tb = ts_idx * TSUP
                for i in range(4):
                    h, tg = divmod(i, 2)
                    hs = slice(CH * h, CH * (h + 1))
                    tsl = slice(tb + tg * 512, tb + (tg + 1) * 512)
                    recip = work.tile([1, 512], F32, tag="recip", name="recip")
                    nc.vector.reciprocal_approx_fast(recip, acp_t[64:65, i, :])
                    rbc = work.tile([64, 512], F32, tag="rbc", name="rbc")
                    nc.gpsimd.partition_broadcast(rbc, recip)
                    nc.vector.tensor_mul(a_cat[hs, tsl], acp_t[0:64, i, :], rbc)

            def emit_proj(ts_idx):
                tb = ts_idx * TSUP
                for m in range(4):
                    ms = slice(m * 128, (m + 1) * 128)
                    for n in range(2):
                        ns = slice(tb + n * 512, tb + (n + 1) * 512)
                        pp = ps.tile([128, 512], F32, tag=f"apl{2 * (n % 2) + m % 2}",
                                     name="pp")
                        nc.tensor.matmul(pp, sb_pw[:, ms], a_cat[:, ns],
                                         start=True, stop=True)
                        pt = work.tile([128, 512], F32, tag="pt", bufs=3, name="pt")
                        nc.vector.tensor_copy(out=pt, in_=pp)
                        nc.sync.dma_start(out=part[ms, ns], in_=pt)

            prev_acp = None
            for tsup in range(NT):
                t0 = tsup * TSUP
                apl = []
                for i in range(4):  # i = 2*h + tg
                    ap_t = ps.tile([65, 512], F32, tag=f"apl{i}", name=f"apl{i}")
                    apl.append(ap_t)
                for j in range(SJ):
                    js = slice(j * 128, (j + 1) * 128)
                    st = (j == 0)
                    sp = (j == SJ - 1)
                    if j == 8 and prev_acp is not None:
                        # normalize the previous stripe mid-loop: DVE/GpSimd are
                        # idle here, and a_cat must be ready before this
                        # stripe's end (proj of prev stripe runs at the next
                        # transition)
                        emit_normalize(tsup - 1, prev_acp)
                    for h in range(HEADS_PER_CORE):
                        vs = slice(65 * h, 65 * (h + 1))
                        sc = ps.tile([128, 1024], F32, tag="sc", bufs=2, name="sc")
                        nc.tensor.matmul(sc[:, 0:512], k2z[h][:, js],
                                         q2[:, t0:t0 + 512], start=True, stop=True)
                        nc.tensor.matmul(sc[:, 512:1024], k2z[h][:, js],
                                         q2[:, t0 + 512:t0 + 1024], start=True, stop=True)
                        E = work.tile([128, 1024], F32R, tag="E", bufs=3, name="E")
                        nc.scalar.activation(out=E, in_=sc, func=AF.Exp, scale=0.125)
                        nc.tensor.matmul(apl[2 * h], vt[:, j, vs], E[:, 0:512],
                                         start=st, stop=sp)
                        nc.tensor.matmul(apl[2 * h + 1], vt[:, j, vs], E[:, 512:1024],
                                         start=st, stop=sp)
                # move a_plus off PSUM quickly (releases the apl banks)
                acp = work.tile([65, 4, 512], F32, tag="acp", bufs=2, name="acp")
                for i in range(4):
                    nc.vector.tensor_copy(out=acp[:, i, :], in_=apl[i])
                # proj of the previous stripe: a_cat was normalized mid-loop
                if tsup > 0:
                    emit_proj(tsup - 1)
                prev_acp = acp
            emit_normalize(NT - 1, prev_acp)
            emit_proj(NT - 1)

    nc.compile()
    return nc


def get_program():
    global _PROGRAM
    if _PROGRAM is None:
        _PROGRAM = build_program()
    return _PROGRAM


def make_in_maps(x, norm_w, norm_b, qkv_w, qkv_b, proj_w):
    """Build the 8 per-core input maps from full inputs."""
    f = np.float32
    x2 = np.ascontiguousarray(x.reshape(B, C, L), dtype=f)

    gmask = np.zeros((128, 4, G), dtype=f)
    bmask = np.zeros((G, 4, 128), dtype=f)
    for t in range(4):
        for p in range(128):
            g = (t * 128 + p) // 16
            gmask[p, t, g] = 1.0 / 16.0
            bmask[g, t, p] = 1.0
    gamma4 = np.ascontiguousarray(norm_w.reshape(4, 128), dtype=f)
    beta4 = np.ascontiguousarray(norm_b.reshape(4, 128), dtype=f)

    in_maps = []
    for cid in range(N_CORES):
        b = cid // 4
        h0 = 2 * (cid % 4)
        h1 = h0 + 1
        qrows = list(range(192 * h0, 192 * h0 + 64)) + list(range(192 * h1, 192 * h1 + 64))
        krows = [r + 64 for r in qrows]
        v0 = list(range(192 * h0 + 128, 192 * h0 + 192))
        v1 = list(range(192 * h1 + 128, 192 * h1 + 192))
        wqT = np.ascontiguousarray(qkv_w[qrows, :].T, dtype=f)
        wkT = np.ascontiguousarray(qkv_w[krows, :].T, dtype=f)
        wvT = np.zeros((C, 256), dtype=f)
        wvT[:, 0:64] = qkv_w[v0, :].T
        wvT[:, 65:129] = qkv_w[v1, :].T
        qbv = np.ascontiguousarray(qkv_b[qrows], dtype=f)
        kbv = np.ascontiguousarray(qkv_b[krows], dtype=f)
        vbv = np.zeros((130,), dtype=f)
        vbv[0:64] = qkv_b[v0]
        vbv[65:129] = qkv_b[v1]
        vbv[64] = 1.0    # softmax-denominator ones columns (weight cols there are 0)
        vbv[129] = 1.0
        ch_cols = list(range(64 * h0, 64 * h0 + 64)) + list(range(64 * h1, 64 * h1 + 64))
        pwT = np.ascontiguousarray(proj_w[:, ch_cols].T, dtype=f)
        in_maps.append({
            "xb": x2[b], "gmask": gmask, "bmask": bmask,
            "gamma4": gamma4, "beta4": beta4,
            "wqT": wqT, "wkT": wkT, "wvT": wvT,
            "qb": qbv, "kb": kbv, "vb": vbv, "pwT": pwT,
        })
    return in_maps


def kernel(x, norm_w, norm_b, qkv_w, qkv_b, proj_w, proj_b, _trace=False):
    x = np.asarray(x, dtype=np.float32)
    in_maps = make_in_maps(x, np.asarray(norm_w), np.asarray(norm_b),
                           np.asarray(qkv_w), np.asarray(qkv_b), np.asarray(proj_w))
    nc = get_program()
    res = run_bass_kernel_spmd(nc, in_maps, list(range(N_CORES)), trace=_trace)
    hout = np.zeros((B, C, L), dtype=np.float32)
    for cid in range(N_CORES):
        hout[cid // 4] += res.results[cid]["part"]
    hout += np.asarray(proj_b, dtype=np.float32)[None, :, None]
    out = x + hout.reshape(x.shape)
    if _trace:
        return out.astype(np.float32), res
    return out.astype(np.float32)


# revision 21
# speedup vs baseline: 1.0067x; 1.0067x over previous
"""AttentionBlock (GroupNorm + QKV + 8-head spatial attention + proj + residual)
on 8 Trainium2 NeuronCores.

Sharding: 16 head-batches (B=2 x NH=8) are split 2-per-core; cores 0-3 take
batch 0, cores 4-7 batch 1.  Each core:
  - loads its batch's x [512, 4096] and computes GroupNorm statistics on-chip
    (bn_stats per channel, group-combine + group->channel broadcast via tiny
    mask matmuls on the PE),
  - folds the GroupNorm affine into the QKV weights (W' = W*A per channel,
    bias' = W@B + qkv_b) so x feeds the QKV matmuls directly,
  - computes q/k for its 2 heads in [c, L] layout and v TRANSPOSED ([L, c])
    straight out of the QKV matmul (x^T @ Wv'^T) so attention needs no
    on-chip transposes,
  - scores are computed in [s, t] layout; softmax denominators come free from
    an extra ones-column in vT (a_plus row 64); exp is done without
    max-subtraction (scores are ~N(0,1) here, exact softmax identity),
  - emits its partial projection  proj_w[:, head_cols] @ a  [512, 4096].
Host sums the 4 partials per batch, adds proj_b and the residual.

All matmuls run as float32r (TF32-like: full PE rate, ~1e-3 worst-case
relative error vs fp32 measured on HW).
"""

import numpy as np

import concourse.bacc as bacc
import concourse.tile as tile
from concourse import mybir
from concourse.bass_utils import run_bass_kernel_spmd

B, C = 2, 512
L = 64 * 64           # 4096
NH = 8                # heads total
CH = 64               # channels per head
G = 32                # groups
EPS = 1e-5
N_CORES = 8
HEADS_PER_CORE = 2

F32 = mybir.dt.float32
F32R = mybir.dt.float32r
AF = mybir.ActivationFunctionType
ALU = mybir.AluOpType

TSUP = 1024           # t-stripe width (2 PSUM banks)
NT = L // TSUP        # 2 stripes
SJ = 32               # number of 128-wide s-chunks


def _f(ap):
    return ap.bitcast(F32)


_PROGRAM = None


def build_program():
    nc = bacc.Bacc()
    xb = nc.declare_dram_parameter("xb", [C, L], F32R, isOutput=False).ap()
    gmask = nc.declare_dram_parameter("gmask", [128, 4, G], F32R, isOutput=False).ap()
    bmask = nc.declare_dram_parameter("bmask", [G, 4, 128], F32R, isOutput=False).ap()
    gamma4 = nc.declare_dram_parameter("gamma4", [4, 128], F32, isOutput=False).ap()
    beta4 = nc.declare_dram_parameter("beta4", [4, 128], F32, isOutput=False).ap()
    wqT = nc.declare_dram_parameter("wqT", [C, 128], F32R, isOutput=False).ap()
    wkT = nc.declare_dram_parameter("wkT", [C, 128], F32R, isOutput=False).ap()
    wvT = nc.declare_dram_parameter("wvT", [C, 256], F32R, isOutput=False).ap()
    qb = nc.declare_dram_parameter("qb", [128], F32, isOutput=False).ap()
    kb = nc.declare_dram_parameter("kb", [128], F32, isOutput=False).ap()
    vb = nc.declare_dram_parameter("vb", [130], F32, isOutput=False).ap()
    pwT = nc.declare_dram_parameter("pwT", [128, C], F32R, isOutput=False).ap()
    part = nc.declare_dram_parameter("part", [C, L], F32, isOutput=True).ap()

    with tile.TileContext(nc) as tc:
        with (
            tc.tile_pool(name="consts", bufs=1) as consts,
            tc.tile_pool(name="big", bufs=1) as big,
            tc.tile_pool(name="work", bufs=2) as work,
            tc.tile_pool(name="ps", bufs=1, space="PSUM") as ps,
        ):
            # ---- constants into SBUF ----
            sb_gmask = consts.tile([128, 4, G], F32R)
            nc.sync.dma_start(out=sb_gmask, in_=gmask)
            sb_bmask = consts.tile([G, 4, 128], F32R)
            nc.sync.dma_start(out=sb_bmask, in_=bmask)
            sb_gamma = consts.tile([128, 4], F32)
            nc.sync.dma_start(out=sb_gamma, in_=gamma4.rearrange("t p -> p t"))
            sb_beta = consts.tile([128, 4], F32)
            nc.sync.dma_start(out=sb_beta, in_=beta4.rearrange("t p -> p t"))
            sb_wq = consts.tile([128, 4, 128], F32R)
            nc.sync.dma_start(out=sb_wq, in_=wqT.rearrange("(kk p) m -> p kk m", p=128))
            sb_wk = consts.tile([128, 4, 128], F32R)
            nc.sync.dma_start(out=sb_wk, in_=wkT.rearrange("(kk p) m -> p kk m", p=128))
            sb_wv = consts.tile([128, 4, 256], F32R)
            nc.sync.dma_start(out=sb_wv, in_=wvT.rearrange("(kk p) m -> p kk m", p=128))
            sb_pw = consts.tile([128, C], F32R)
            nc.sync.dma_start(out=sb_pw, in_=pwT)
            sb_qb = consts.tile([128, 1], F32)
            nc.sync.dma_start(out=sb_qb, in_=qb.unsqueeze(1))
            sb_kb = consts.tile([128, 1], F32)
            nc.sync.dma_start(out=sb_kb, in_=kb.unsqueeze(1))
            sb_vb = consts.tile([1, 130], F32)
            nc.sync.dma_start(out=sb_vb, in_=vb.unsqueeze(0))
            eps32 = consts.tile([32, 1], F32)
            nc.vector.memset(eps32, EPS)
            mh0 = consts.tile([128, 1], F32)
            nc.vector.memset(mh0[0:64, :], 1.0)
            nc.vector.memset(mh0[64:128, :], 0.0)
            mh1 = consts.tile([128, 1], F32)
            nc.vector.memset(mh1[0:64, :], 0.0)
            nc.vector.memset(mh1[64:128, :], 1.0)

            # ---- load x ----
            xt = big.tile([128, 4, L], F32R)
            xbr = xb.rearrange("(t p) l -> p t l", p=128)
            stats = work.tile([128, 4, 8, 6], F32, bufs=1)
            for s in range(8):
                ns = slice(s * 512, (s + 1) * 512)
                nc.sync.dma_start(out=xt[:, :, ns], in_=xbr[:, :, ns])
                for t in range(4):
                    nc.vector.bn_stats(
                        out=stats[:, t, s, :], in_=_f(xt[:, t, ns])
                    )
            mv = work.tile([128, 4, 2], F32, bufs=1)
            for t in range(4):
                nc.vector.bn_aggr(out=mv[:, t, :], in_=stats[:, t, :, :])
            # per-channel [mean, var+mean^2]
            stats2 = work.tile([128, 4, 2], F32R, bufs=1)
            msq = work.tile([128, 4, 1], F32, bufs=1)
            nc.vector.tensor_copy(out=stats2[:, :, 0:1], in_=mv[:, :, 0:1])
            nc.vector.tensor_mul(msq, mv[:, :, 0:1], mv[:, :, 0:1])
            nc.vector.tensor_add(stats2[:, :, 1:2], mv[:, :, 1:2], msq)
            # group stats via mask matmul: [32, 2] = (mean_g, E[x^2]_g)
            gps = ps.tile([32, 2], F32, tag="apl0")
            for t in range(4):
                nc.tensor.matmul(
                    gps, sb_gmask[:, t, :], stats2[:, t, :],
                    start=(t == 0), stop=(t == 3),
                )
            gs = work.tile([32, 2], F32, bufs=1)
            nc.vector.tensor_copy(out=gs, in_=gps)
            msqg = work.tile([32, 1], F32, bufs=1)
            varg = work.tile([32, 1], F32, bufs=1)
            nc.vector.tensor_mul(msqg, gs[:, 0:1], gs[:, 0:1])
            nc.vector.tensor_sub(varg, gs[:, 1:2], msqg)
            # rstd = exp(-0.5*ln(var+eps))  (Ln+Exp share one ACT table set)
            lng = work.tile([32, 1], F32, bufs=1)
            nc.scalar.activation(out=lng, in_=varg, func=AF.Ln, bias=eps32, scale=1.0)
            rstdg = work.tile([32, 1], F32, bufs=1)
            nc.scalar.activation(out=rstdg, in_=lng, func=AF.Exp, scale=-0.5)
            gstats2 = work.tile([32, 2], F32R, bufs=1)
            nc.vector.tensor_copy(out=gstats2[:, 0:1], in_=gs[:, 0:1])
            nc.vector.tensor_copy(out=gstats2[:, 1:2], in_=rstdg)

            # ---- per-channel affine A, Bs  (hid = x*A + Bs) ----
            A_all = work.tile([128, 4], F32, bufs=1)
            Bcol = work.tile([128, 4, 2], F32R, bufs=1)
            for t in range(4):
                cst = ps.tile([128, 2], F32, tag="apl1")
                nc.tensor.matmul(
                    cst, sb_bmask[:, t, :], gstats2, start=True, stop=True
                )
                nc.vector.tensor_mul(A_all[:, t:t + 1], cst[:, 1:2], sb_gamma[:, t:t + 1])
                tmp = work.tile([128, 1], F32, tag="tmp")
                nc.vector.tensor_mul(tmp, cst[:, 0:1], A_all[:, t:t + 1])
                nc.vector.tensor_sub(Bcol[:, t, :], sb_beta[:, t:t + 1].broadcast_to([128, 2]), tmp.broadcast_to([128, 2]))

            # ---- fold affine into QKV weights ----
            # bias' = W^T @ Bs + b first (reads original W), then W *= A in place
            cq_ps = ps.tile([128, 2], F32, tag="sc", bufs=2)
            ck_ps = ps.tile([128, 2], F32, tag="apl2")
            cv_ps = ps.tile([1, 256], F32, tag="apl0")
            for t in range(4):
                nc.tensor.matmul(cq_ps, sb_wq[:, t, :], Bcol[:, t, :],
                                 start=(t == 0), stop=(t == 3))
                nc.tensor.matmul(ck_ps, sb_wk[:, t, :], Bcol[:, t, :],
                                 start=(t == 0), stop=(t == 3))
                nc.tensor.matmul(cv_ps, Bcol[:, t, 0:1], sb_wv[:, t, :],
                                 start=(t == 0), stop=(t == 3))
            qc = consts.tile([128, 1], F32)
            nc.vector.tensor_add(qc, cq_ps[:, 0:1], sb_qb)
            kc = consts.tile([128, 1], F32)
            nc.vector.tensor_add(kc, ck_ps[:, 0:1], sb_kb)
            vrow = work.tile([1, 130], F32, bufs=1)
            nc.vector.tensor_add(vrow, cv_ps[:, 0:130], sb_vb)
            vbc = consts.tile([128, 130], F32)
            nc.gpsimd.partition_broadcast(vbc, vrow)
            for t in range(4):
                nc.vector.tensor_scalar_mul(
                    out=sb_wq[:, t, :], in0=_f(sb_wq[:, t, :]), scalar1=A_all[:, t:t + 1])
                nc.vector.tensor_scalar_mul(
                    out=sb_wk[:, t, :], in0=_f(sb_wk[:, t, :]), scalar1=A_all[:, t:t + 1])
                nc.vector.tensor_scalar_mul(
                    out=sb_wv[:, t, :], in0=_f(sb_wv[:, t, :]), scalar1=A_all[:, t:t + 1])

            # ---- QKV ----
            q2 = big.tile([128, L], F32R)
            k2z = [big.tile([128, L], F32R, name="k2z0"),
                   big.tile([128, L], F32R, name="k2z1")]
            for n in range(8):
                ns = slice(n * 512, (n + 1) * 512)
                qp = ps.tile([128, 512], F32, tag="sc", bufs=2, name="qp")
                for kk in range(4):
                    nc.tensor.matmul(qp, sb_wq[:, kk, :], xt[:, kk, ns],
                                     start=(kk == 0), stop=(kk == 3))
                nc.vector.tensor_scalar_add(out=q2[:, ns], in0=qp, scalar1=qc)
                kp = ps.tile([128, 512], F32, tag=("apl2" if n % 2 == 0 else "apl3"), name="kp")
                for kk in range(4):
                    nc.tensor.matmul(kp, sb_wk[:, kk, :], xt[:, kk, ns],
                                     start=(kk == 0), stop=(kk == 3))
                # (k + kc) masked per head: other head's partitions zeroed so the
                # scores matmul can contract over all 128 partitions (K=128 is
                # 2x faster than K=64 for f32r)
                nc.vector.tensor_scalar(out=k2z[0][:, ns], in0=kp, scalar1=kc,
                                        scalar2=mh0, op0=ALU.add, op1=ALU.mult)
                nc.vector.tensor_scalar(out=k2z[1][:, ns], in0=kp, scalar1=kc,
                                        scalar2=mh1, op0=ALU.add, op1=ALU.mult)
            # vT: [s, c] both heads + ones cols at 64 (h0) / 129 (h1)
            vt = big.tile([128, SJ, 130], F32R)
            for j in range(SJ):
                js = slice(j * 128, (j + 1) * 128)
                vp = ps.tile([128, 256], F32, tag=("apl0" if j % 2 == 0 else "apl1"), name="vp")
                for kk in range(4):
                    nc.tensor.matmul(vp, xt[:, kk, js], sb_wv[:, kk, :],
                                     start=(kk == 0), stop=(kk == 3))
                nc.vector.tensor_add(vt[:, j, 0:130], vp[:, 0:130], vbc)

            # ---- attention ----
            # Both heads interleaved per j: two independent scores->exp chains
            # keep ACT (the bottleneck) saturated while the PE stays warm.
            a_cat = big.tile([128, L], F32R, tag="xt")

            def emit_normalize(ts_idx, acp_t):
                tb = ts_idx * TSUP
                for i in range(4):
                    h, tg = divmod(i, 2)
                    hs = slice(CH * h, CH * (h + 1))
                    tsl = slice(tb + tg * 512, tb + (tg + 1) * 512)
                    recip = work.tile([1, 512], F32, tag="recip", name="recip")
                    nc.vector.reciprocal_approx_fast(recip, acp_t[64:65, i, :])
                    rbc = work.tile([64, 512], F32, tag="rbc", name="rbc")
                    nc.gpsimd.partition_broadcast(rbc, recip)
                    nc.vector.tensor_mul(a_cat[hs, tsl], acp_t[0:64, i, :], rbc)

            def emit_proj(ts_idx):
                tb = ts_idx * TSUP
                for m in range(4):
                    ms = slice(m * 128, (m + 1) * 128)
                    for n in range(2):
                        ns = slice(tb + n * 512, tb + (n + 1) * 512)
                        pp = ps.tile([128, 512], F32, tag=f"apl{2 * (n % 2) + m % 2}",
                                     name="pp")
                        nc.tensor.matmul(pp, sb_pw[:, ms], a_cat[:, ns],
                                         start=True, stop=True)
                        pt = work.tile([128, 512], F32, tag="pt", bufs=3, name="pt")
                        nc.vector.tensor_copy(out=pt, in_=pp)
                        nc.sync.dma_start(out=part[ms, ns], in_=pt)

            prev_acp = None
            for tsup in range(NT):
                t0 = tsup * TSUP
                apl = []
                for i in range(4):  # i = 2*h + tg
                    ap_t = ps.tile([65, 512], F32, tag=f"apl{i}", name=f"apl{i}")
                    apl.append(ap_t)
                for j in range(SJ):
                    js = slice(j * 128, (j + 1) * 128)
                    st = (j == 0)
                    sp = (j == SJ - 1)
                    if j == 8 and prev_acp is not None:
                        # normalize the previous stripe mid-loop: DVE/GpSimd are
                        # idle here, and a_cat must be ready before this
                        # stripe's end (proj of prev stripe runs at the next
                        # transition)
                        emit_normalize(tsup - 1, prev_acp)
                    Es = []
                    for h in range(HEADS_PER_CORE):
                        sc = ps.tile([128, 1024], F32, tag="sc", bufs=2, name="sc")
                        nc.tensor.matmul(sc[:, 0:512], k2z[h][:, js],
                                         q2[:, t0:t0 + 512], start=True, stop=True)
                        nc.tensor.matmul(sc[:, 512:1024], k2z[h][:, js],
                                         q2[:, t0 + 512:t0 + 1024], start=True, stop=True)
                        E = work.tile([128, 1024], F32R, tag="E", bufs=3, name="E")
                        nc.scalar.activation(out=E, in_=sc, func=AF.Exp, scale=0.125)
                        Es.append(E)
                    for h in range(HEADS_PER_CORE):
                        vs = slice(65 * h, 65 * (h + 1))
                        nc.tensor.matmul(apl[2 * h], vt[:, j, vs], Es[h][:, 0:512],
                                         start=st, stop=sp)
                        nc.tensor.matmul(apl[2 * h + 1], vt[:, j, vs], Es[h][:, 512:1024],
                                         start=st, stop=sp)
                # move a_plus off PSUM quickly (releases the apl banks)
                acp = work.tile([65, 4, 512], F32, tag="acp", bufs=2, name="acp")
                for i in range(4):
                    nc.vector.tensor_copy(out=acp[:, i, :], in_=apl[i])
                # proj of the previous stripe: a_cat was normalized mid-loop
                if tsup > 0:
                    emit_proj(tsup - 1)
                prev_acp = acp
            emit_normalize(NT - 1, prev_acp)
            emit_proj(NT - 1)

    nc.compile()
    return nc


def get_program():
    global _PROGRAM
    if _PROGRAM is None:
        _PROGRAM = build_program()
    return _PROGRAM


def make_in_maps(x, norm_w, norm_b, qkv_w, qkv_b, proj_w):
    """Build the 8 per-core input maps from full inputs."""
    f = np.float32
    x2 = np.ascontiguousarray(x.reshape(B, C, L), dtype=f)

    gmask = np.zeros((128, 4, G), dtype=f)
    bmask = np.zeros((G, 4, 128), dtype=f)
    for t in range(4):
        for p in range(128):
            g = (t * 128 + p) // 16
            gmask[p, t, g] = 1.0 / 16.0
            bmask[g, t, p] = 1.0
    gamma4 = np.ascontiguousarray(norm_w.reshape(4, 128), dtype=f)
    beta4 = np.ascontiguousarray(norm_b.reshape(4, 128), dtype=f)

    in_maps = []
    for cid in range(N_CORES):
        b = cid // 4
        h0 = 2 * (cid % 4)
        h1 = h0 + 1
        qrows = list(range(192 * h0, 192 * h0 + 64)) + list(range(192 * h1, 192 * h1 + 64))
        krows = [r + 64 for r in qrows]
        v0 = list(range(192 * h0 + 128, 192 * h0 + 192))
        v1 = list(range(192 * h1 + 128, 192 * h1 + 192))
        wqT = np.ascontiguousarray(qkv_w[qrows, :].T, dtype=f)
        wkT = np.ascontiguousarray(qkv_w[krows, :].T, dtype=f)
        wvT = np.zeros((C, 256), dtype=f)
        wvT[:, 0:64] = qkv_w[v0, :].T
        wvT[:, 65:129] = qkv_w[v1, :].T
        qbv = np.ascontiguousarray(qkv_b[qrows], dtype=f)
        kbv = np.ascontiguousarray(qkv_b[krows], dtype=f)
        vbv = np.zeros((130,), dtype=f)
        vbv[0:64] = qkv_b[v0]
        vbv[65:129] = qkv_b[v1]
        vbv[64] = 1.0    # softmax-denominator ones columns (weight cols there are 0)
        vbv[129] = 1.0
        ch_cols = list(range(64 * h0, 64 * h0 + 64)) + list(range(64 * h1, 64 * h1 + 64))
        pwT = np.ascontiguousarray(proj_w[:, ch_cols].T, dtype=f)
        in_maps.append({
            "xb": x2[b], "gmask": gmask, "bmask": bmask,
            "gamma4": gamma4, "beta4": beta4,
            "wqT": wqT, "wkT": wkT, "wvT": wvT,
            "qb": qbv, "kb": kbv, "vb": vbv, "pwT": pwT,
        })
    return in_maps


def kernel(x, norm_w, norm_b, qkv_w, qkv_b, proj_w, proj_b, _trace=False):
    x = np.asarray(x, dtype=np.float32)
    in_maps = make_in_maps(x, np.asarray(norm_w), np.asarray(norm_b),
                           np.asarray(qkv_w), np.asarray(qkv_b), np.asarray(proj_w))
    nc = get_program()
    res = run_bass_kernel_spmd(nc, in_maps, list(range(N_CORES)), trace=_trace)
    hout = np.zeros((B, C, L), dtype=np.float32)
    for cid in range(N_CORES):
        hout[cid // 4] += res.results[cid]["part"]
    hout += np.asarray(proj_b, dtype=np.float32)[None, :, None]
    out = x + hout.reshape(x.shape)
    if _trace:
        return out.astype(np.float32), res
    return out.astype(np.float32)


# revision 22
# speedup vs baseline: 1.0912x; 1.0839x over previous
"""AttentionBlock (GroupNorm + QKV + 8-head spatial attention + proj + residual)
on 8 Trainium2 NeuronCores.

Sharding: 16 head-batches (B=2 x NH=8) are split 2-per-core; cores 0-3 take
batch 0, cores 4-7 batch 1.  Each core:
  - loads its batch's x [512, 4096] and computes GroupNorm statistics on-chip
    (bn_stats per channel, group-combine + group->channel broadcast via tiny
    mask matmuls on the PE),
  - folds the GroupNorm affine into the QKV weights (W' = W*A per channel,
    bias' = W@B + qkv_b) so x feeds the QKV matmuls directly,
  - computes q/k for its 2 heads in [c, L] layout and v TRANSPOSED ([L, c])
    straight out of the QKV matmul (x^T @ Wv'^T) so attention needs no
    on-chip transposes,
  - scores are computed in [s, t] layout; softmax denominators come free from
    an extra ones-column in vT (a_plus row 64); exp is done without
    max-subtraction (scores are ~N(0,1) here, exact softmax identity),
  - emits its partial projection  proj_w[:, head_cols] @ a  [512, 4096].
Host sums the 4 partials per batch, adds proj_b and the residual.

All matmuls run as float32r (TF32-like: full PE rate, ~1e-3 worst-case
relative error vs fp32 measured on HW).
"""

import numpy as np

import concourse.bacc as bacc
import concourse.tile as tile
from concourse import mybir
from concourse.bass_utils import run_bass_kernel_spmd

B, C = 2, 512
L = 64 * 64           # 4096
NH = 8                # heads total
CH = 64               # channels per head
G = 32                # groups
EPS = 1e-5
N_CORES = 8
HEADS_PER_CORE = 2

F32 = mybir.dt.float32
F32R = mybir.dt.float32r
AF = mybir.ActivationFunctionType
ALU = mybir.AluOpType

TSUP = 1024           # t-stripe width (2 PSUM banks)
NT = L // TSUP        # 2 stripes
SJ = 32               # number of 128-wide s-chunks


def _f(ap):
    return ap.bitcast(F32)


_PROGRAM = None


def build_program():
    nc = bacc.Bacc()
    xb = nc.declare_dram_parameter("xb", [C, L], F32R, isOutput=False).ap()
    gmask = nc.declare_dram_parameter("gmask", [128, 4, G], F32R, isOutput=False).ap()
    bmask = nc.declare_dram_parameter("bmask", [G, 4, 128], F32R, isOutput=False).ap()
    gamma4 = nc.declare_dram_parameter("gamma4", [4, 128], F32, isOutput=False).ap()
    beta4 = nc.declare_dram_parameter("beta4", [4, 128], F32, isOutput=False).ap()
    wqT = nc.declare_dram_parameter("wqT", [C, 128], F32R, isOutput=False).ap()
    wkT = nc.declare_dram_parameter("wkT", [C, 128], F32R, isOutput=False).ap()
    wvT = nc.declare_dram_parameter("wvT", [C, 256], F32R, isOutput=False).ap()
    qb = nc.declare_dram_parameter("qb", [128], F32, isOutput=False).ap()
    kb = nc.declare_dram_parameter("kb", [128], F32, isOutput=False).ap()
    vb = nc.declare_dram_parameter("vb", [130], F32, isOutput=False).ap()
    pwT = nc.declare_dram_parameter("pwT", [128, C], F32R, isOutput=False).ap()
    part = nc.declare_dram_parameter("part", [C, L], F32, isOutput=True).ap()

    with tile.TileContext(nc) as tc:
        with (
            tc.tile_pool(name="consts", bufs=1) as consts,
            tc.tile_pool(name="big", bufs=1) as big,
            tc.tile_pool(name="work", bufs=2) as work,
            tc.tile_pool(name="ps", bufs=1, space="PSUM") as ps,
        ):
            # ---- constants into SBUF ----
            sb_gmask = consts.tile([128, 4, G], F32R)
            nc.sync.dma_start(out=sb_gmask, in_=gmask)
            sb_bmask = consts.tile([G, 4, 128], F32R)
            nc.sync.dma_start(out=sb_bmask, in_=bmask)
            sb_gamma = consts.tile([128, 4], F32)
            nc.sync.dma_start(out=sb_gamma, in_=gamma4.rearrange("t p -> p t"))
            sb_beta = consts.tile([128, 4], F32)
            nc.sync.dma_start(out=sb_beta, in_=beta4.rearrange("t p -> p t"))
            sb_wq = consts.tile([128, 4, 128], F32R)
            nc.sync.dma_start(out=sb_wq, in_=wqT.rearrange("(kk p) m -> p kk m", p=128))
            sb_wk = consts.tile([128, 4, 128], F32R)
            nc.sync.dma_start(out=sb_wk, in_=wkT.rearrange("(kk p) m -> p kk m", p=128))
            sb_wv = consts.tile([128, 4, 256], F32R)
            nc.sync.dma_start(out=sb_wv, in_=wvT.rearrange("(kk p) m -> p kk m", p=128))
            sb_pw = consts.tile([128, C], F32R)
            nc.sync.dma_start(out=sb_pw, in_=pwT)
            sb_qb = consts.tile([128, 1], F32)
            nc.sync.dma_start(out=sb_qb, in_=qb.unsqueeze(1))
            sb_kb = consts.tile([128, 1], F32)
            nc.sync.dma_start(out=sb_kb, in_=kb.unsqueeze(1))
            sb_vb = consts.tile([1, 130], F32)
            nc.sync.dma_start(out=sb_vb, in_=vb.unsqueeze(0))
            eps32 = consts.tile([32, 1], F32)
            nc.vector.memset(eps32, EPS)
            mh0 = consts.tile([128, 1], F32)
            nc.vector.memset(mh0[0:64, :], 1.0)
            nc.vector.memset(mh0[64:128, :], 0.0)
            mh1 = consts.tile([128, 1], F32)
            nc.vector.memset(mh1[0:64, :], 0.0)
            nc.vector.memset(mh1[64:128, :], 1.0)

            # ---- load x ----
            xt = big.tile([128, 4, L], F32R)
            xbr = xb.rearrange("(t p) l -> p t l", p=128)
            stats = work.tile([128, 4, 8, 6], F32, bufs=1)
            for s in range(8):
                ns = slice(s * 512, (s + 1) * 512)
                nc.sync.dma_start(out=xt[:, :, ns], in_=xbr[:, :, ns])
                for t in range(4):
                    nc.vector.bn_stats(
                        out=stats[:, t, s, :], in_=_f(xt[:, t, ns])
                    )
            mv = work.tile([128, 4, 2], F32, bufs=1)
            for t in range(4):
                nc.vector.bn_aggr(out=mv[:, t, :], in_=stats[:, t, :, :])
            # per-channel [mean, var+mean^2]
            stats2 = work.tile([128, 4, 2], F32R, bufs=1)
            msq = work.tile([128, 4, 1], F32, bufs=1)
            nc.vector.tensor_copy(out=stats2[:, :, 0:1], in_=mv[:, :, 0:1])
            nc.vector.tensor_mul(msq, mv[:, :, 0:1], mv[:, :, 0:1])
            nc.vector.tensor_add(stats2[:, :, 1:2], mv[:, :, 1:2], msq)
            # group stats via mask matmul: [32, 2] = (mean_g, E[x^2]_g)
            gps = ps.tile([32, 2], F32, tag="apl0")
            for t in range(4):
                nc.tensor.matmul(
                    gps, sb_gmask[:, t, :], stats2[:, t, :],
                    start=(t == 0), stop=(t == 3),
                )
            gs = work.tile([32, 2], F32, bufs=1)
            nc.vector.tensor_copy(out=gs, in_=gps)
            msqg = work.tile([32, 1], F32, bufs=1)
            varg = work.tile([32, 1], F32, bufs=1)
            nc.vector.tensor_mul(msqg, gs[:, 0:1], gs[:, 0:1])
            nc.vector.tensor_sub(varg, gs[:, 1:2], msqg)
            # rstd = exp(-0.5*ln(var+eps))  (Ln+Exp share one ACT table set)
            lng = work.tile([32, 1], F32, bufs=1)
            nc.scalar.activation(out=lng, in_=varg, func=AF.Ln, bias=eps32, scale=1.0)
            rstdg = work.tile([32, 1], F32, bufs=1)
            nc.scalar.activation(out=rstdg, in_=lng, func=AF.Exp, scale=-0.5)
            gstats2 = work.tile([32, 2], F32R, bufs=1)
            nc.vector.tensor_copy(out=gstats2[:, 0:1], in_=gs[:, 0:1])
            nc.vector.tensor_copy(out=gstats2[:, 1:2], in_=rstdg)

            # ---- per-channel affine A, Bs  (hid = x*A + Bs) ----
            A_all = work.tile([128, 4], F32, bufs=1)
            Bcol = work.tile([128, 4, 2], F32R, bufs=1)
            for t in range(4):
                cst = ps.tile([128, 2], F32, tag="apl1")
                nc.tensor.matmul(
                    cst, sb_bmask[:, t, :], gstats2, start=True, stop=True
                )
                nc.vector.tensor_mul(A_all[:, t:t + 1], cst[:, 1:2], sb_gamma[:, t:t + 1])
                tmp = work.tile([128, 1], F32, tag="tmp")
                nc.vector.tensor_mul(tmp, cst[:, 0:1], A_all[:, t:t + 1])
                nc.vector.tensor_sub(Bcol[:, t, :], sb_beta[:, t:t + 1].broadcast_to([128, 2]), tmp.broadcast_to([128, 2]))

            # ---- fold affine into QKV weights ----
            # bias' = W^T @ Bs + b first (reads original W), then W *= A in place
            cq_ps = ps.tile([128, 2], F32, tag="sc", bufs=2)
            ck_ps = ps.tile([128, 2], F32, tag="apl2")
            cv_ps = ps.tile([1, 256], F32, tag="apl0")
            for t in range(4):
                nc.tensor.matmul(cq_ps, sb_wq[:, t, :], Bcol[:, t, :],
                                 start=(t == 0), stop=(t == 3))
                nc.tensor.matmul(ck_ps, sb_wk[:, t, :], Bcol[:, t, :],
                                 start=(t == 0), stop=(t == 3))
                nc.tensor.matmul(cv_ps, Bcol[:, t, 0:1], sb_wv[:, t, :],
                                 start=(t == 0), stop=(t == 3))
            qc = consts.tile([128, 1], F32)
            nc.vector.tensor_add(qc, cq_ps[:, 0:1], sb_qb)
            kc = consts.tile([128, 1], F32)
            nc.vector.tensor_add(kc, ck_ps[:, 0:1], sb_kb)
            vrow = work.tile([1, 130], F32, bufs=1)
            nc.vector.tensor_add(vrow, cv_ps[:, 0:130], sb_vb)
            vbc = consts.tile([128, 130], F32)
            nc.gpsimd.partition_broadcast(vbc, vrow)
            for t in range(4):
                nc.vector.tensor_scalar_mul(
                    out=sb_wq[:, t, :], in0=_f(sb_wq[:, t, :]), scalar1=A_all[:, t:t + 1])
                nc.vector.tensor_scalar_mul(
                    out=sb_wk[:, t, :], in0=_f(sb_wk[:, t, :]), scalar1=A_all[:, t:t + 1])
                nc.vector.tensor_scalar_mul(
                    out=sb_wv[:, t, :], in0=_f(sb_wv[:, t, :]), scalar1=A_all[:, t:t + 1])

            # ---- QKV ----
            q2 = big.tile([128, L], F32R)
            k2z = [big.tile([128, L], F32R, name="k2z0"),
                   big.tile([128, L], F32R, name="k2z1")]
            for n in range(8):
                ns = slice(n * 512, (n + 1) * 512)
                qp = ps.tile([128, 512], F32, tag="sc", bufs=2, name="qp")
                for kk in range(4):
                    nc.tensor.matmul(qp, sb_wq[:, kk, :], xt[:, kk, ns],
                                     start=(kk == 0), stop=(kk == 3))
                nc.vector.tensor_scalar_add(out=q2[:, ns], in0=qp, scalar1=qc)
                kp = ps.tile([128, 512], F32, tag=("apl2" if n % 2 == 0 else "apl3"), name="kp")
                for kk in range(4):
                    nc.tensor.matmul(kp, sb_wk[:, kk, :], xt[:, kk, ns],
                                     start=(kk == 0), stop=(kk == 3))
                # (k + kc) masked per head: other head's partitions zeroed so the
                # scores matmul can contract over all 128 partitions (K=128 is
                # 2x faster than K=64 for f32r)
                nc.vector.tensor_scalar(out=k2z[0][:, ns], in0=kp, scalar1=kc,
                                        scalar2=mh0, op0=ALU.add, op1=ALU.mult)
                nc.vector.tensor_scalar(out=k2z[1][:, ns], in0=kp, scalar1=kc,
                                        scalar2=mh1, op0=ALU.add, op1=ALU.mult)
            # vT: [s, c] both heads + ones cols at 64 (h0) / 129 (h1)
            vt = big.tile([128, SJ, 130], F32R)
            for j in range(SJ):
                js = slice(j * 128, (j + 1) * 128)
                vp = ps.tile([128, 256], F32, tag=("apl0" if j % 2 == 0 else "apl1"), name="vp")
                for kk in range(4):
                    nc.tensor.matmul(vp, xt[:, kk, js], sb_wv[:, kk, :],
                                     start=(kk == 0), stop=(kk == 3))
                nc.vector.tensor_add(vt[:, j, 0:130], vp[:, 0:130], vbc)

            # ---- attention ----
            # Both heads interleaved per j: two independent scores->exp chains
            # keep ACT (the bottleneck) saturated while the PE stays warm.
            a_cat = big.tile([128, L], F32R, tag="xt")

            def emit_normalize(ts_idx, acp_t):
                tb = ts_idx * TSUP
                for i in range(4):
                    h, tg = divmod(i, 2)
                    hs = slice(CH * h, CH * (h + 1))
                    tsl = slice(tb + tg * 512, tb + (tg + 1) * 512)
                    recip = work.tile([1, 512], F32, tag="recip", name="recip")
                    nc.vector.reciprocal_approx_fast(recip, acp_t[64:65, i, :])
                    rbc = work.tile([64, 512], F32, tag="rbc", name="rbc")
                    nc.gpsimd.partition_broadcast(rbc, recip)
                    nc.vector.tensor_mul(a_cat[hs, tsl], acp_t[0:64, i, :], rbc)

            def emit_proj(ts_idx):
                tb = ts_idx * TSUP
                for m in range(4):
                    ms = slice(m * 128, (m + 1) * 128)
                    for n in range(2):
                        ns = slice(tb + n * 512, tb + (n + 1) * 512)
                        pp = ps.tile([128, 512], F32, tag=f"apl{2 * (n % 2) + m % 2}",
                                     name="pp")
                        nc.tensor.matmul(pp, sb_pw[:, ms], a_cat[:, ns],
                                         start=True, stop=True)
                        pt = work.tile([128, 512], F32, tag="pt", bufs=2, name="pt")
                        nc.vector.tensor_copy(out=pt, in_=pp)
                        nc.sync.dma_start(out=part[ms, ns], in_=pt)

            prev_acp = None
            for tsup in range(NT):
                t0 = tsup * TSUP
                apl = []
                for i in range(4):  # i = 2*h + tg
                    ap_t = ps.tile([65, 512], F32, tag=f"apl{i}", name=f"apl{i}")
                    apl.append(ap_t)
                prevE = None
                for j in range(SJ + 1):
                    if j == 8 and prev_acp is not None:
                        # normalize the previous stripe mid-loop: DVE/GpSimd are
                        # idle here, and a_cat must be ready before this
                        # stripe's end (proj of prev stripe runs at the next
                        # transition)
                        emit_normalize(tsup - 1, prev_acp)
                    Es = []
                    if j < SJ:
                        js = slice(j * 128, (j + 1) * 128)
                        for h in range(HEADS_PER_CORE):
                            sc = ps.tile([128, 1024], F32, tag="sc", bufs=2, name="sc")
                            nc.tensor.matmul(sc[:, 0:512], k2z[h][:, js],
                                             q2[:, t0:t0 + 512], start=True, stop=True)
                            nc.tensor.matmul(sc[:, 512:1024], k2z[h][:, js],
                                             q2[:, t0 + 512:t0 + 1024], start=True, stop=True)
                            E = work.tile([128, 1024], F32R, tag="E", bufs=4, name="E")
                            nc.scalar.activation(out=E, in_=sc, func=AF.Exp, scale=0.125)
                            Es.append(E)
                    # avs lag one j so the PE never stalls waiting for exp
                    if prevE is not None:
                        jj = j - 1
                        pjs = slice(jj * 128, (jj + 1) * 128)
                        st = (jj == 0)
                        sp = (jj == SJ - 1)
                        for h in range(HEADS_PER_CORE):
                            vs = slice(65 * h, 65 * (h + 1))
                            nc.tensor.matmul(apl[2 * h], vt[:, jj, vs],
                                             prevE[h][:, 0:512], start=st, stop=sp)
                            nc.tensor.matmul(apl[2 * h + 1], vt[:, jj, vs],
                                             prevE[h][:, 512:1024], start=st, stop=sp)
                    prevE = Es if j < SJ else None
                # move a_plus off PSUM quickly (releases the apl banks)
                acp = work.tile([65, 4, 512], F32, tag="acp", bufs=2, name="acp")
                for i in range(4):
                    nc.vector.tensor_copy(out=acp[:, i, :], in_=apl[i])
                # proj of the previous stripe: a_cat was normalized mid-loop
                if tsup > 0:
                    emit_proj(tsup - 1)
                prev_acp = acp
            emit_normalize(NT - 1, prev_acp)
            emit_proj(NT - 1)

    nc.compile()
    return nc


def get_program():
    global _PROGRAM
    if _PROGRAM is None:
        _PROGRAM = build_program()
    return _PROGRAM


def make_in_maps(x, norm_w, norm_b, qkv_w, qkv_b, proj_w):
    """Build the 8 per-core input maps from full inputs."""
    f = np.float32
    x2 = np.ascontiguousarray(x.reshape(B, C, L), dtype=f)

    gmask = np.zeros((128, 4, G), dtype=f)
    bmask = np.zeros((G, 4, 128), dtype=f)
    for t in range(4):
        for p in range(128):
            g = (t * 128 + p) // 16
            gmask[p, t, g] = 1.0 / 16.0
            bmask[g, t, p] = 1.0
    gamma4 = np.ascontiguousarray(norm_w.reshape(4, 128), dtype=f)
    beta4 = np.ascontiguousarray(norm_b.reshape(4, 128), dtype=f)

    in_maps = []
    for cid in range(N_CORES):
        b = cid // 4
        h0 = 2 * (cid % 4)
        h1 = h0 + 1
        qrows = list(range(192 * h0, 192 * h0 + 64)) + list(range(192 * h1, 192 * h1 + 64))
        krows = [r + 64 for r in qrows]
        v0 = list(range(192 * h0 + 128, 192 * h0 + 192))
        v1 = list(range(192 * h1 + 128, 192 * h1 + 192))
        wqT = np.ascontiguousarray(qkv_w[qrows, :].T, dtype=f)
        wkT = np.ascontiguousarray(qkv_w[krows, :].T, dtype=f)
        wvT = np.zeros((C, 256), dtype=f)
        wvT[:, 0:64] = qkv_w[v0, :].T
        wvT[:, 65:129] = qkv_w[v1, :].T
        qbv = np.ascontiguousarray(qkv_b[qrows], dtype=f)
        kbv = np.ascontiguousarray(qkv_b[krows], dtype=f)
        vbv = np.zeros((130,), dtype=f)
        vbv[0:64] = qkv_b[v0]
        vbv[65:129] = qkv_b[v1]
        vbv[64] = 1.0    # softmax-denominator ones columns (weight cols there are 0)
        vbv[129] = 1.0
        ch_cols = list(range(64 * h0, 64 * h0 + 64)) + list(range(64 * h1, 64 * h1 + 64))
        pwT = np.ascontiguousarray(proj_w[:, ch_cols].T, dtype=f)
        in_maps.append({
            "xb": x2[b], "gmask": gmask, "bmask": bmask,
            "gamma4": gamma4, "beta4": beta4,
            "wqT": wqT, "wkT": wkT, "wvT": wvT,
            "qb": qbv, "kb": kbv, "vb": vbv, "pwT": pwT,
        })
    return in_maps


def kernel(x, norm_w, norm_b, qkv_w, qkv_b, proj_w, proj_b, _trace=False):
    x = np.asarray(x, dtype=np.float32)
    in_maps = make_in_maps(x, np.asarray(norm_w), np.asarray(norm_b),
                           np.asarray(qkv_w), np.asarray(qkv_b), np.asarray(proj_w))
    nc = get_program()
    res = run_bass_kernel_spmd(nc, in_maps, list(range(N_CORES)), trace=_trace)
    hout = np.zeros((B, C, L), dtype=np.float32)
    for cid in range(N_CORES):
        hout[cid // 4] += res.results[cid]["part"]
    hout += np.asarray(proj_b, dtype=np.float32)[None, :, None]
    out = x + hout.reshape(x.shape)
    if _trace:
        return out.astype(np.float32), res
    return out.astype(np.float32)


# revision 26
# speedup vs baseline: 1.1753x; 1.0771x over previous
"""AttentionBlock (GroupNorm + QKV + 8-head spatial attention + proj + residual)
on 8 Trainium2 NeuronCores.

Sharding: 16 head-batches (B=2 x NH=8) are split 2-per-core; cores 0-3 take
batch 0, cores 4-7 batch 1.  Each core:
  - loads its batch's x [512, 4096] and computes GroupNorm statistics on-chip
    (bn_stats per channel, group-combine + group->channel broadcast via tiny
    mask matmuls on the PE),
  - folds the GroupNorm affine into the QKV weights (W' = W*A per channel,
    bias' = W@B + qkv_b) so x feeds the QKV matmuls directly,
  - computes q/k for its 2 heads in [c, L] layout and v TRANSPOSED ([L, c])
    straight out of the QKV matmul (x^T @ Wv'^T) so attention needs no
    on-chip transposes,
  - scores are computed in [s, t] layout; softmax denominators come free from
    an extra ones-column in vT (a_plus row 64); exp is done without
    max-subtraction (scores are ~N(0,1) here, exact softmax identity),
  - emits its partial projection  proj_w[:, head_cols] @ a  [512, 4096].
Host sums the 4 partials per batch, adds proj_b and the residual.

All matmuls run as float32r (TF32-like: full PE rate, ~1e-3 worst-case
relative error vs fp32 measured on HW).
"""

import numpy as np

import concourse.bacc as bacc
import concourse.tile as tile
from concourse import mybir
from concourse.bass_utils import run_bass_kernel_spmd

B, C = 2, 512
L = 64 * 64           # 4096
NH = 8                # heads total
CH = 64               # channels per head
G = 32                # groups
EPS = 1e-5
N_CORES = 8
HEADS_PER_CORE = 2

F32 = mybir.dt.float32
F32R = mybir.dt.float32r
AF = mybir.ActivationFunctionType
ALU = mybir.AluOpType

TSUP = 1024           # t-stripe width (2 PSUM banks)
NT = L // TSUP        # 2 stripes
SJ = 32               # number of 128-wide s-chunks


def _f(ap):
    return ap.bitcast(F32)


_PROGRAM = None


def build_program():
    nc = bacc.Bacc()
    xb = nc.declare_dram_parameter("xb", [C, L], F32R, isOutput=False).ap()
    gmask = nc.declare_dram_parameter("gmask", [128, 4, G], F32R, isOutput=False).ap()
    bmask = nc.declare_dram_parameter("bmask", [G, 4, 128], F32R, isOutput=False).ap()
    gamma4 = nc.declare_dram_parameter("gamma4", [4, 128], F32, isOutput=False).ap()
    beta4 = nc.declare_dram_parameter("beta4", [4, 128], F32, isOutput=False).ap()
    wqT = nc.declare_dram_parameter("wqT", [C, 128], F32R, isOutput=False).ap()
    wkT = nc.declare_dram_parameter("wkT", [C, 128], F32R, isOutput=False).ap()
    wvT = nc.declare_dram_parameter("wvT", [C, 256], F32R, isOutput=False).ap()
    qb = nc.declare_dram_parameter("qb", [128], F32, isOutput=False).ap()
    kb = nc.declare_dram_parameter("kb", [128], F32, isOutput=False).ap()
    vb = nc.declare_dram_parameter("vb", [130], F32, isOutput=False).ap()
    pwT = nc.declare_dram_parameter("pwT", [128, C], F32R, isOutput=False).ap()
    part = nc.declare_dram_parameter("part", [C, L], F32, isOutput=True).ap()

    with tile.TileContext(nc) as tc:
        with (
            tc.tile_pool(name="consts", bufs=1) as consts,
            tc.tile_pool(name="big", bufs=1) as big,
            tc.tile_pool(name="work", bufs=2) as work,
            tc.tile_pool(name="ps", bufs=1, space="PSUM") as ps,
        ):
            # ---- constants into SBUF ----
            sb_gmask = consts.tile([128, 4, G], F32R)
            nc.sync.dma_start(out=sb_gmask, in_=gmask)
            sb_bmask = consts.tile([G, 4, 128], F32R)
            nc.sync.dma_start(out=sb_bmask, in_=bmask)
            sb_gamma = consts.tile([128, 4], F32)
            nc.sync.dma_start(out=sb_gamma, in_=gamma4.rearrange("t p -> p t"))
            sb_beta = consts.tile([128, 4], F32)
            nc.sync.dma_start(out=sb_beta, in_=beta4.rearrange("t p -> p t"))
            sb_wq = consts.tile([128, 4, 128], F32R)
            nc.sync.dma_start(out=sb_wq, in_=wqT.rearrange("(kk p) m -> p kk m", p=128))
            sb_wk = consts.tile([128, 4, 128], F32R)
            nc.sync.dma_start(out=sb_wk, in_=wkT.rearrange("(kk p) m -> p kk m", p=128))
            sb_wv = consts.tile([128, 4, 256], F32R)
            nc.sync.dma_start(out=sb_wv, in_=wvT.rearrange("(kk p) m -> p kk m", p=128))
            sb_pw = consts.tile([128, C], F32R)
            nc.sync.dma_start(out=sb_pw, in_=pwT)
            sb_qb = consts.tile([128, 1], F32)
            nc.sync.dma_start(out=sb_qb, in_=qb.unsqueeze(1))
            sb_kb = consts.tile([128, 1], F32)
            nc.sync.dma_start(out=sb_kb, in_=kb.unsqueeze(1))
            sb_vb = consts.tile([1, 130], F32)
            nc.sync.dma_start(out=sb_vb, in_=vb.unsqueeze(0))
            eps32 = consts.tile([32, 1], F32)
            nc.vector.memset(eps32, EPS)
            mh0 = consts.tile([128, 1], F32)
            nc.vector.memset(mh0[0:64, :], 1.0)
            nc.vector.memset(mh0[64:128, :], 0.0)
            mh1 = consts.tile([128, 1], F32)
            nc.vector.memset(mh1[0:64, :], 0.0)
            nc.vector.memset(mh1[64:128, :], 1.0)

            # ---- load x ----
            xt = big.tile([128, 4, L], F32R)
            xbr = xb.rearrange("(t p) l -> p t l", p=128)
            stats = work.tile([128, 4, 8, 6], F32, bufs=1)
            for s in range(8):
                ns = slice(s * 512, (s + 1) * 512)
                nc.sync.dma_start(out=xt[:, :, ns], in_=xbr[:, :, ns])
                for t in range(4):
                    nc.vector.bn_stats(
                        out=stats[:, t, s, :], in_=_f(xt[:, t, ns])
                    )
            mv = work.tile([128, 4, 2], F32, bufs=1)
            for t in range(4):
                nc.vector.bn_aggr(out=mv[:, t, :], in_=stats[:, t, :, :])
            # per-channel [mean, var+mean^2]
            stats2 = work.tile([128, 4, 2], F32R, bufs=1)
            msq = work.tile([128, 4, 1], F32, bufs=1)
            nc.vector.tensor_copy(out=stats2[:, :, 0:1], in_=mv[:, :, 0:1])
            nc.vector.tensor_mul(msq, mv[:, :, 0:1], mv[:, :, 0:1])
            nc.vector.tensor_add(stats2[:, :, 1:2], mv[:, :, 1:2], msq)
            # group stats via mask matmul: [32, 2] = (mean_g, E[x^2]_g)
            gps = ps.tile([32, 2], F32, tag="apl0")
            for t in range(4):
                nc.tensor.matmul(
                    gps, sb_gmask[:, t, :], stats2[:, t, :],
                    start=(t == 0), stop=(t == 3),
                )
            gs = work.tile([32, 2], F32, bufs=1)
            nc.vector.tensor_copy(out=gs, in_=gps)
            msqg = work.tile([32, 1], F32, bufs=1)
            varg = work.tile([32, 1], F32, bufs=1)
            nc.vector.tensor_mul(msqg, gs[:, 0:1], gs[:, 0:1])
            nc.vector.tensor_sub(varg, gs[:, 1:2], msqg)
            # rstd = exp(-0.5*ln(var+eps))  (Ln+Exp share one ACT table set)
            lng = work.tile([32, 1], F32, bufs=1)
            nc.scalar.activation(out=lng, in_=varg, func=AF.Ln, bias=eps32, scale=1.0)
            rstdg = work.tile([32, 1], F32, bufs=1)
            nc.scalar.activation(out=rstdg, in_=lng, func=AF.Exp, scale=-0.5)
            gstats2 = work.tile([32, 2], F32R, bufs=1)
            nc.vector.tensor_copy(out=gstats2[:, 0:1], in_=gs[:, 0:1])
            nc.vector.tensor_copy(out=gstats2[:, 1:2], in_=rstdg)

            # ---- per-channel affine A, Bs  (hid = x*A + Bs) ----
            A_all = work.tile([128, 4], F32, bufs=1)
            Bcol = work.tile([128, 4, 2], F32R, bufs=1)
            for t in range(4):
                cst = ps.tile([128, 2], F32, tag="apl1")
                nc.tensor.matmul(
                    cst, sb_bmask[:, t, :], gstats2, start=True, stop=True
                )
                nc.vector.tensor_mul(A_all[:, t:t + 1], cst[:, 1:2], sb_gamma[:, t:t + 1])
                tmp = work.tile([128, 1], F32, tag="tmp")
                nc.vector.tensor_mul(tmp, cst[:, 0:1], A_all[:, t:t + 1])
                nc.vector.tensor_sub(Bcol[:, t, :], sb_beta[:, t:t + 1].broadcast_to([128, 2]), tmp.broadcast_to([128, 2]))

            # ---- fold affine into QKV weights ----
            # bias' = W^T @ Bs + b first (reads original W), then W *= A in place
            cq_ps = ps.tile([128, 2], F32, tag="sc", bufs=2)
            ck_ps = ps.tile([128, 2], F32, tag="apl0")
            cv_ps = ps.tile([1, 256], F32, tag="apl0")
            for t in range(4):
                nc.tensor.matmul(cq_ps, sb_wq[:, t, :], Bcol[:, t, :],
                                 start=(t == 0), stop=(t == 3))
                nc.tensor.matmul(ck_ps, sb_wk[:, t, :], Bcol[:, t, :],
                                 start=(t == 0), stop=(t == 3))
                nc.tensor.matmul(cv_ps, Bcol[:, t, 0:1], sb_wv[:, t, :],
                                 start=(t == 0), stop=(t == 3))
            qc = consts.tile([128, 1], F32)
            nc.vector.tensor_add(qc, cq_ps[:, 0:1], sb_qb)
            kc = consts.tile([128, 1], F32)
            nc.vector.tensor_add(kc, ck_ps[:, 0:1], sb_kb)
            vrow = work.tile([1, 130], F32, bufs=1)
            nc.vector.tensor_add(vrow, cv_ps[:, 0:130], sb_vb)
            vbc = consts.tile([128, 130], F32)
            nc.gpsimd.partition_broadcast(vbc, vrow)
            for t in range(4):
                nc.vector.tensor_scalar_mul(
                    out=sb_wq[:, t, :], in0=_f(sb_wq[:, t, :]), scalar1=A_all[:, t:t + 1])
                nc.vector.tensor_scalar_mul(
                    out=sb_wk[:, t, :], in0=_f(sb_wk[:, t, :]), scalar1=A_all[:, t:t + 1])
                nc.vector.tensor_scalar_mul(
                    out=sb_wv[:, t, :], in0=_f(sb_wv[:, t, :]), scalar1=A_all[:, t:t + 1])

            # ---- QKV ----
            q2 = big.tile([128, L], F32R)
            k2z = [big.tile([128, L], F32R, name="k2z0"),
                   big.tile([128, L], F32R, name="k2z1")]
            for n in range(8):
                ns = slice(n * 512, (n + 1) * 512)
                qp = ps.tile([128, 512], F32, tag="sc", bufs=2, name="qp")
                for kk in range(4):
                    nc.tensor.matmul(qp, sb_wq[:, kk, :], xt[:, kk, ns],
                                     start=(kk == 0), stop=(kk == 3))
                nc.vector.tensor_scalar_add(out=q2[:, ns], in0=qp, scalar1=qc)
                kp = ps.tile([128, 512], F32, tag="pp", bufs=2, name="kp")
                for kk in range(4):
                    nc.tensor.matmul(kp, sb_wk[:, kk, :], xt[:, kk, ns],
                                     start=(kk == 0), stop=(kk == 3))
                # (k + kc) masked per head: other head's partitions zeroed so the
                # scores matmul can contract over all 128 partitions (K=128 is
                # 2x faster than K=64 for f32r)
                nc.vector.tensor_scalar(out=k2z[0][:, ns], in0=kp, scalar1=kc,
                                        scalar2=mh0, op0=ALU.add, op1=ALU.mult)
                nc.vector.tensor_scalar(out=k2z[1][:, ns], in0=kp, scalar1=kc,
                                        scalar2=mh1, op0=ALU.add, op1=ALU.mult)
            # vT: [s, c] both heads + ones cols at 64 (h0) / 129 (h1)
            vt = big.tile([128, SJ, 130], F32R)
            for j in range(SJ):
                js = slice(j * 128, (j + 1) * 128)
                vp = ps.tile([128, 256], F32, tag=("apl0" if j % 2 == 0 else "apl1"), name="vp")
                for kk in range(4):
                    nc.tensor.matmul(vp, xt[:, kk, js], sb_wv[:, kk, :],
                                     start=(kk == 0), stop=(kk == 3))
                nc.vector.tensor_add(vt[:, j, 0:130], vp[:, 0:130], vbc)

            # ---- attention ----
            # Per (h, tsup) stripe of 1024 t-columns.  Scores go to a
            # double-buffered 2-bank PSUM tile; exp (ACT) is the bottleneck and
            # runs back-to-back; the a_plus accumulation (av) lags one j so the
            # in-order PE stream never stalls waiting for an exp.  Projection
            # of each stripe is emitted one stripe later (its inputs are then
            # long-ready) on its own PSUM banks.
            a_cat = big.tile([128, L], F32R, tag="xt")

            def emit_normalize(key, acp_t):
                hh, ts_idx = key
                tb = ts_idx * TSUP
                hsn = slice(CH * hh, CH * (hh + 1))
                for tg in range(2):
                    tsl = slice(tb + tg * 512, tb + (tg + 1) * 512)
                    recip = work.tile([1, 512], F32, tag="recip", name="recip")
                    nc.vector.reciprocal(recip, acp_t[64:65, tg, :])
                    rbc = work.tile([64, 512], F32, tag="rbc", name="rbc")
                    nc.gpsimd.partition_broadcast(rbc, recip)
                    nc.vector.tensor_mul(a_cat[hsn, tsl], acp_t[0:64, tg, :], rbc)

            def emit_proj(ts_idx):
                # needs a_cat rows of BOTH heads for this t-range
                tb = ts_idx * TSUP
                for m in range(4):
                    ms = slice(m * 128, (m + 1) * 128)
                    for n in range(2):
                        ns = slice(tb + n * 512, tb + (n + 1) * 512)
                        pp = ps.tile([128, 512], F32, tag="pp", bufs=2, name="pp")
                        nc.tensor.matmul(pp, sb_pw[:, ms], a_cat[:, ns],
                                         start=True, stop=True)
                        pt = work.tile([128, 512], F32, tag="pt", bufs=2, name="pt")
                        nc.vector.tensor_copy(out=pt, in_=pp)
                        nc.sync.dma_start(out=part[ms, ns], in_=pt)

            pending_norm = None   # (key, acp) not yet normalized
            for tsup in range(NT):
                t0 = tsup * TSUP
                for h in range(HEADS_PER_CORE):
                    vs = slice(65 * h, 65 * (h + 1))
                    apl = []
                    for tg in range(2):
                        ap_t = ps.tile([65, 512], F32, tag=f"apl{tg}", name=f"apl{tg}")
                        apl.append(ap_t)
                    prevE = None
                    for j in range(SJ + 1):
                        if j == 8 and pending_norm is not None:
                            emit_normalize(*pending_norm)
                            pending_norm = None
                        if j == 12 and h == 1 and tsup > 0:
                            # project the previous t-stripe mid-loop (PE has
                            # slack; inputs long-ready; own PSUM banks)
                            emit_proj(tsup - 1)
                        if j < SJ:
                            js = slice(j * 128, (j + 1) * 128)
                            sc = ps.tile([128, 1024], F32, tag="sc", bufs=2, name="sc")
                            nc.tensor.matmul(sc[:, 0:512], k2z[h][:, js],
                                             q2[:, t0:t0 + 512], start=True, stop=True)
                            nc.tensor.matmul(sc[:, 512:1024], k2z[h][:, js],
                                             q2[:, t0 + 512:t0 + 1024],
                                             start=True, stop=True)
                            E = work.tile([128, 1024], F32R, tag="E", bufs=4, name="E")
                            nc.scalar.activation(out=E, in_=sc, func=AF.Exp, scale=0.125)
                        if prevE is not None:
                            jj = j - 1
                            st = (jj == 0)
                            sp = (jj == SJ - 1)
                            nc.tensor.matmul(apl[0], vt[:, jj, vs],
                                             prevE[:, 0:512], start=st, stop=sp)
                            nc.tensor.matmul(apl[1], vt[:, jj, vs],
                                             prevE[:, 512:1024], start=st, stop=sp)
                        prevE = E if j < SJ else None
                    # move a_plus off PSUM quickly (releases the apl banks)
                    acp = work.tile([65, 2, 512], F32, tag="acp", bufs=3, name="acp")
                    for tg in range(2):
                        nc.vector.tensor_copy(out=acp[:, tg, :], in_=apl[tg])
                    pending_norm = ((h, tsup), acp)
            emit_normalize(*pending_norm)
            emit_proj(NT - 1)

    nc.compile()
    return nc


def get_program():
    global _PROGRAM
    if _PROGRAM is None:
        _PROGRAM = build_program()
    return _PROGRAM


def make_in_maps(x, norm_w, norm_b, qkv_w, qkv_b, proj_w):
    """Build the 8 per-core input maps from full inputs."""
    f = np.float32
    x2 = np.ascontiguousarray(x.reshape(B, C, L), dtype=f)

    gmask = np.zeros((128, 4, G), dtype=f)
    bmask = np.zeros((G, 4, 128), dtype=f)
    for t in range(4):
        for p in range(128):
            g = (t * 128 + p) // 16
            gmask[p, t, g] = 1.0 / 16.0
            bmask[g, t, p] = 1.0
    gamma4 = np.ascontiguousarray(norm_w.reshape(4, 128), dtype=f)
    beta4 = np.ascontiguousarray(norm_b.reshape(4, 128), dtype=f)

    in_maps = []
    for cid in range(N_CORES):
        b = cid // 4
        h0 = 2 * (cid % 4)
        h1 = h0 + 1
        qrows = list(range(192 * h0, 192 * h0 + 64)) + list(range(192 * h1, 192 * h1 + 64))
        krows = [r + 64 for r in qrows]
        v0 = list(range(192 * h0 + 128, 192 * h0 + 192))
        v1 = list(range(192 * h1 + 128, 192 * h1 + 192))
        wqT = np.ascontiguousarray(qkv_w[qrows, :].T, dtype=f)
        wkT = np.ascontiguousarray(qkv_w[krows, :].T, dtype=f)
        wvT = np.zeros((C, 256), dtype=f)
        wvT[:, 0:64] = qkv_w[v0, :].T
        wvT[:, 65:129] = qkv_w[v1, :].T
        qbv = np.ascontiguousarray(qkv_b[qrows], dtype=f)
        kbv = np.ascontiguousarray(qkv_b[krows], dtype=f)
        vbv = np.zeros((130,), dtype=f)
        vbv[0:64] = qkv_b[v0]
        vbv[65:129] = qkv_b[v1]
        vbv[64] = 1.0    # softmax-denominator ones columns (weight cols there are 0)
        vbv[129] = 1.0
        ch_cols = list(range(64 * h0, 64 * h0 + 64)) + list(range(64 * h1, 64 * h1 + 64))
        pwT = np.ascontiguousarray(proj_w[:, ch_cols].T, dtype=f)
        in_maps.append({
            "xb": x2[b], "gmask": gmask, "bmask": bmask,
            "gamma4": gamma4, "beta4": beta4,
            "wqT": wqT, "wkT": wkT, "wvT": wvT,
            "qb": qbv, "kb": kbv, "vb": vbv, "pwT": pwT,
        })
    return in_maps


def kernel(x, norm_w, norm_b, qkv_w, qkv_b, proj_w, proj_b, _trace=False):
    x = np.asarray(x, dtype=np.float32)
    in_maps = make_in_maps(x, np.asarray(norm_w), np.asarray(norm_b),
                           np.asarray(qkv_w), np.asarray(qkv_b), np.asarray(proj_w))
    nc = get_program()
    res = run_bass_kernel_spmd(nc, in_maps, list(range(N_CORES)), trace=_trace)
    hout = np.zeros((B, C, L), dtype=np.float32)
    for cid in range(N_CORES):
        hout[cid // 4] += res.results[cid]["part"]
    hout += np.asarray(proj_b, dtype=np.float32)[None, :, None]
    out = x + hout.reshape(x.shape)
    if _trace:
        return out.astype(np.float32), res
    return out.astype(np.float32)


# revision 27
# speedup vs baseline: 1.2108x; 1.0303x over previous
"""AttentionBlock (GroupNorm + QKV + 8-head spatial attention + proj + residual)
on 8 Trainium2 NeuronCores.

Sharding: 16 head-batches (B=2 x NH=8) are split 2-per-core; cores 0-3 take
batch 0, cores 4-7 batch 1.  Each core:
  - loads its batch's x [512, 4096] and computes GroupNorm statistics on-chip
    (bn_stats per channel, group-combine + group->channel broadcast via tiny
    mask matmuls on the PE),
  - folds the GroupNorm affine into the QKV weights (W' = W*A per channel,
    bias' = W@B + qkv_b) so x feeds the QKV matmuls directly,
  - computes q/k for its 2 heads in [c, L] layout and v TRANSPOSED ([L, c])
    straight out of the QKV matmul (x^T @ Wv'^T) so attention needs no
    on-chip transposes,
  - scores are computed in [s, t] layout; softmax denominators come free from
    an extra ones-column in vT (a_plus row 64); exp is done without
    max-subtraction (scores are ~N(0,1) here, exact softmax identity),
  - emits its partial projection  proj_w[:, head_cols] @ a  [512, 4096].
Host sums the 4 partials per batch, adds proj_b and the residual.

All matmuls run as float32r (TF32-like: full PE rate, ~1e-3 worst-case
relative error vs fp32 measured on HW).
"""

import numpy as np

import concourse.bacc as bacc
import concourse.tile as tile
from concourse import mybir
from concourse.bass_utils import run_bass_kernel_spmd

B, C = 2, 512
L = 64 * 64           # 4096
NH = 8                # heads total
CH = 64               # channels per head
G = 32                # groups
EPS = 1e-5
N_CORES = 8
HEADS_PER_CORE = 2

F32 = mybir.dt.float32
F32R = mybir.dt.float32r
AF = mybir.ActivationFunctionType
ALU = mybir.AluOpType

TSUP = 1024           # t-stripe width (2 PSUM banks)
NT = L // TSUP        # 2 stripes
SJ = 32               # number of 128-wide s-chunks


def _f(ap):
    return ap.bitcast(F32)


_PROGRAM = None


def build_program():
    nc = bacc.Bacc()
    xb = nc.declare_dram_parameter("xb", [C, L], F32R, isOutput=False).ap()
    gmask = nc.declare_dram_parameter("gmask", [128, 4, G], F32R, isOutput=False).ap()
    bmask = nc.declare_dram_parameter("bmask", [G, 4, 128], F32R, isOutput=False).ap()
    gamma4 = nc.declare_dram_parameter("gamma4", [4, 128], F32, isOutput=False).ap()
    beta4 = nc.declare_dram_parameter("beta4", [4, 128], F32, isOutput=False).ap()
    wqT = nc.declare_dram_parameter("wqT", [C, 128], F32R, isOutput=False).ap()
    wkT = nc.declare_dram_parameter("wkT", [C, 128], F32R, isOutput=False).ap()
    wvT = nc.declare_dram_parameter("wvT", [C, 256], F32R, isOutput=False).ap()
    qb = nc.declare_dram_parameter("qb", [128], F32, isOutput=False).ap()
    kb = nc.declare_dram_parameter("kb", [128], F32, isOutput=False).ap()
    vb = nc.declare_dram_parameter("vb", [130], F32, isOutput=False).ap()
    pwT = nc.declare_dram_parameter("pwT", [128, C], F32R, isOutput=False).ap()
    part = nc.declare_dram_parameter("part", [C, L], F32, isOutput=True).ap()

    with tile.TileContext(nc) as tc:
        with (
            tc.tile_pool(name="consts", bufs=1) as consts,
            tc.tile_pool(name="big", bufs=1) as big,
            tc.tile_pool(name="work", bufs=2) as work,
            tc.tile_pool(name="ps", bufs=1, space="PSUM") as ps,
        ):
            # ---- constants into SBUF ----
            sb_gmask = consts.tile([128, 4, G], F32R)
            nc.sync.dma_start(out=sb_gmask, in_=gmask)
            sb_bmask = consts.tile([G, 4, 128], F32R)
            nc.sync.dma_start(out=sb_bmask, in_=bmask)
            sb_gamma = consts.tile([128, 4], F32)
            nc.sync.dma_start(out=sb_gamma, in_=gamma4.rearrange("t p -> p t"))
            sb_beta = consts.tile([128, 4], F32)
            nc.sync.dma_start(out=sb_beta, in_=beta4.rearrange("t p -> p t"))
            sb_wq = consts.tile([128, 4, 128], F32R)
            nc.sync.dma_start(out=sb_wq, in_=wqT.rearrange("(kk p) m -> p kk m", p=128))
            sb_wk = consts.tile([128, 4, 128], F32R)
            nc.sync.dma_start(out=sb_wk, in_=wkT.rearrange("(kk p) m -> p kk m", p=128))
            sb_wv = consts.tile([128, 4, 256], F32R)
            nc.sync.dma_start(out=sb_wv, in_=wvT.rearrange("(kk p) m -> p kk m", p=128))
            sb_pw = consts.tile([128, C], F32R)
            nc.sync.dma_start(out=sb_pw, in_=pwT)
            sb_qb = consts.tile([128, 1], F32)
            nc.sync.dma_start(out=sb_qb, in_=qb.unsqueeze(1))
            sb_kb = consts.tile([128, 1], F32)
            nc.sync.dma_start(out=sb_kb, in_=kb.unsqueeze(1))
            sb_vb = consts.tile([1, 130], F32)
            nc.sync.dma_start(out=sb_vb, in_=vb.unsqueeze(0))
            eps32 = consts.tile([32, 1], F32)
            nc.vector.memset(eps32, EPS)
            mh0 = consts.tile([128, 1], F32)
            nc.vector.memset(mh0[0:64, :], 1.0)
            nc.vector.memset(mh0[64:128, :], 0.0)
            mh1 = consts.tile([128, 1], F32)
            nc.vector.memset(mh1[0:64, :], 0.0)
            nc.vector.memset(mh1[64:128, :], 1.0)

            # ---- load x ----
            xt = big.tile([128, 4, L], F32R)
            xbr = xb.rearrange("(t p) l -> p t l", p=128)
            stats = work.tile([128, 4, 8, 6], F32, bufs=1)
            for s in range(8):
                ns = slice(s * 512, (s + 1) * 512)
                nc.sync.dma_start(out=xt[:, :, ns], in_=xbr[:, :, ns])
                for t in range(4):
                    nc.vector.bn_stats(
                        out=stats[:, t, s, :], in_=_f(xt[:, t, ns])
                    )
            mv = work.tile([128, 4, 2], F32, bufs=1)
            for t in range(4):
                nc.vector.bn_aggr(out=mv[:, t, :], in_=stats[:, t, :, :])
            # per-channel [mean, var+mean^2]
            stats2 = work.tile([128, 4, 2], F32R, bufs=1)
            msq = work.tile([128, 4, 1], F32, bufs=1)
            nc.vector.tensor_copy(out=stats2[:, :, 0:1], in_=mv[:, :, 0:1])
            nc.vector.tensor_mul(msq, mv[:, :, 0:1], mv[:, :, 0:1])
            nc.vector.tensor_add(stats2[:, :, 1:2], mv[:, :, 1:2], msq)
            # group stats via mask matmul: [32, 2] = (mean_g, E[x^2]_g)
            gps = ps.tile([32, 2], F32, tag="apl0")
            for t in range(4):
                nc.tensor.matmul(
                    gps, sb_gmask[:, t, :], stats2[:, t, :],
                    start=(t == 0), stop=(t == 3),
                )
            gs = work.tile([32, 2], F32, bufs=1)
            nc.vector.tensor_copy(out=gs, in_=gps)
            msqg = work.tile([32, 1], F32, bufs=1)
            varg = work.tile([32, 1], F32, bufs=1)
            nc.vector.tensor_mul(msqg, gs[:, 0:1], gs[:, 0:1])
            nc.vector.tensor_sub(varg, gs[:, 1:2], msqg)
            # rstd = exp(-0.5*ln(var+eps))  (Ln+Exp share one ACT table set)
            lng = work.tile([32, 1], F32, bufs=1)
            nc.scalar.activation(out=lng, in_=varg, func=AF.Ln, bias=eps32, scale=1.0)
            rstdg = work.tile([32, 1], F32, bufs=1)
            nc.scalar.activation(out=rstdg, in_=lng, func=AF.Exp, scale=-0.5)
            gstats2 = work.tile([32, 2], F32R, bufs=1)
            nc.vector.tensor_copy(out=gstats2[:, 0:1], in_=gs[:, 0:1])
            nc.vector.tensor_copy(out=gstats2[:, 1:2], in_=rstdg)

            # ---- per-channel affine A, Bs  (hid = x*A + Bs) ----
            A_all = work.tile([128, 4], F32, bufs=1)
            Bcol = work.tile([128, 4, 2], F32R, bufs=1)
            for t in range(4):
                cst = ps.tile([128, 2], F32, tag="apl1")
                nc.tensor.matmul(
                    cst, sb_bmask[:, t, :], gstats2, start=True, stop=True
                )
                nc.vector.tensor_mul(A_all[:, t:t + 1], cst[:, 1:2], sb_gamma[:, t:t + 1])
                tmp = work.tile([128, 1], F32, tag="tmp")
                nc.vector.tensor_mul(tmp, cst[:, 0:1], A_all[:, t:t + 1])
                nc.vector.tensor_sub(Bcol[:, t, :], sb_beta[:, t:t + 1].broadcast_to([128, 2]), tmp.broadcast_to([128, 2]))

            # ---- fold affine into QKV weights ----
            # bias' = W^T @ Bs + b first (reads original W), then W *= A in place
            cq_ps = ps.tile([128, 2], F32, tag="sc", bufs=2)
            ck_ps = ps.tile([128, 2], F32, tag="apl0")
            cv_ps = ps.tile([1, 256], F32, tag="apl0")
            for t in range(4):
                nc.tensor.matmul(cq_ps, sb_wq[:, t, :], Bcol[:, t, :],
                                 start=(t == 0), stop=(t == 3))
                nc.tensor.matmul(ck_ps, sb_wk[:, t, :], Bcol[:, t, :],
                                 start=(t == 0), stop=(t == 3))
                nc.tensor.matmul(cv_ps, Bcol[:, t, 0:1], sb_wv[:, t, :],
                                 start=(t == 0), stop=(t == 3))
            qc = consts.tile([128, 1], F32)
            nc.vector.tensor_add(qc, cq_ps[:, 0:1], sb_qb)
            kc = consts.tile([128, 1], F32)
            nc.vector.tensor_add(kc, ck_ps[:, 0:1], sb_kb)
            vrow = work.tile([1, 130], F32, bufs=1)
            nc.vector.tensor_add(vrow, cv_ps[:, 0:130], sb_vb)
            vbc = consts.tile([128, 130], F32)
            nc.gpsimd.partition_broadcast(vbc, vrow)
            for t in range(4):
                nc.vector.tensor_scalar_mul(
                    out=sb_wq[:, t, :], in0=_f(sb_wq[:, t, :]), scalar1=A_all[:, t:t + 1])
                nc.vector.tensor_scalar_mul(
                    out=sb_wk[:, t, :], in0=_f(sb_wk[:, t, :]), scalar1=A_all[:, t:t + 1])
                nc.vector.tensor_scalar_mul(
                    out=sb_wv[:, t, :], in0=_f(sb_wv[:, t, :]), scalar1=A_all[:, t:t + 1])

            # ---- QKV ----
            q2 = big.tile([128, L], F32R)
            k2z = [big.tile([128, L], F32R, name="k2z0"),
                   big.tile([128, L], F32R, name="k2z1")]
            for n in range(8):
                ns = slice(n * 512, (n + 1) * 512)
                qp = ps.tile([128, 512], F32, tag="sc", bufs=2, name="qp")
                for kk in range(4):
                    nc.tensor.matmul(qp, sb_wq[:, kk, :], xt[:, kk, ns],
                                     start=(kk == 0), stop=(kk == 3))
                nc.vector.tensor_scalar_add(out=q2[:, ns], in0=qp, scalar1=qc)
                kp = ps.tile([128, 512], F32, tag="pp", bufs=2, name="kp")
                for kk in range(4):
                    nc.tensor.matmul(kp, sb_wk[:, kk, :], xt[:, kk, ns],
                                     start=(kk == 0), stop=(kk == 3))
                # (k + kc) masked per head: other head's partitions zeroed so the
                # scores matmul can contract over all 128 partitions (K=128 is
                # 2x faster than K=64 for f32r)
                nc.vector.tensor_scalar(out=k2z[0][:, ns], in0=kp, scalar1=kc,
                                        scalar2=mh0, op0=ALU.add, op1=ALU.mult)
                nc.vector.tensor_scalar(out=k2z[1][:, ns], in0=kp, scalar1=kc,
                                        scalar2=mh1, op0=ALU.add, op1=ALU.mult)
            # vT: [s, c] both heads + ones cols at 64 (h0) / 129 (h1).
            # Produced inside stripe (0,0)'s j-loop: one s-chunk per j, one j
            # ahead of its consumption by the lagged avs.
            vt = big.tile([128, SJ, 130], F32R)

            def emit_vp(j):
                js = slice(j * 128, (j + 1) * 128)
                vp = ps.tile([128, 256], F32, tag="pp", bufs=2, name="vp")
                for kk in range(4):
                    nc.tensor.matmul(vp, xt[:, kk, js], sb_wv[:, kk, :],
                                     start=(kk == 0), stop=(kk == 3))
                nc.vector.tensor_add(vt[:, j, 0:130], vp[:, 0:130], vbc)

            # ---- attention ----
            # Per (h, tsup) stripe of 1024 t-columns.  Scores go to a
            # double-buffered 2-bank PSUM tile; exp (ACT) is the bottleneck and
            # runs back-to-back; the a_plus accumulation (av) lags one j so the
            # in-order PE stream never stalls waiting for an exp.  Projection
            # of each stripe is emitted one stripe later (its inputs are then
            # long-ready) on its own PSUM banks.
            a_cat = big.tile([128, L], F32R, tag="xt")

            def emit_normalize(key, acp_t):
                hh, ts_idx = key
                tb = ts_idx * TSUP
                hsn = slice(CH * hh, CH * (hh + 1))
                for tg in range(2):
                    tsl = slice(tb + tg * 512, tb + (tg + 1) * 512)
                    recip = work.tile([1, 512], F32, tag="recip", name="recip")
                    nc.vector.reciprocal(recip, acp_t[64:65, tg, :])
                    rbc = work.tile([64, 512], F32, tag="rbc", name="rbc")
                    nc.gpsimd.partition_broadcast(rbc, recip)
                    nc.vector.tensor_mul(a_cat[hsn, tsl], acp_t[0:64, tg, :], rbc)

            def emit_proj(ts_idx):
                # needs a_cat rows of BOTH heads for this t-range
                tb = ts_idx * TSUP
                for m in range(4):
                    ms = slice(m * 128, (m + 1) * 128)
                    for n in range(2):
                        ns = slice(tb + n * 512, tb + (n + 1) * 512)
                        pp = ps.tile([128, 512], F32, tag="pp", bufs=2, name="pp")
                        nc.tensor.matmul(pp, sb_pw[:, ms], a_cat[:, ns],
                                         start=True, stop=True)
                        pt = work.tile([128, 512], F32, tag="pt", bufs=2, name="pt")
                        nc.vector.tensor_copy(out=pt, in_=pp)
                        nc.sync.dma_start(out=part[ms, ns], in_=pt)

            pending_norm = None   # (key, acp) not yet normalized
            for tsup in range(NT):
                t0 = tsup * TSUP
                for h in range(HEADS_PER_CORE):
                    vs = slice(65 * h, 65 * (h + 1))
                    apl = []
                    for tg in range(2):
                        ap_t = ps.tile([65, 512], F32, tag=f"apl{tg}", name=f"apl{tg}")
                        apl.append(ap_t)
                    prevE = None
                    for j in range(SJ + 1):
                        if j == 8 and pending_norm is not None:
                            emit_normalize(*pending_norm)
                            pending_norm = None
                        if j == 12 and h == 1 and tsup > 0:
                            # project the previous t-stripe mid-loop (PE has
                            # slack; inputs long-ready; own PSUM banks)
                            emit_proj(tsup - 1)
                        if tsup == 0 and h == 0 and j < SJ:
                            emit_vp(j)
                        if j < SJ:
                            js = slice(j * 128, (j + 1) * 128)
                            sc = ps.tile([128, 1024], F32, tag="sc", bufs=2, name="sc")
                            nc.tensor.matmul(sc[:, 0:512], k2z[h][:, js],
                                             q2[:, t0:t0 + 512], start=True, stop=True)
                            nc.tensor.matmul(sc[:, 512:1024], k2z[h][:, js],
                                             q2[:, t0 + 512:t0 + 1024],
                                             start=True, stop=True)
                            E = work.tile([128, 1024], F32R, tag="E", bufs=4, name="E")
                            nc.scalar.activation(out=E, in_=sc, func=AF.Exp, scale=0.125)
                        if prevE is not None:
                            jj = j - 1
                            st = (jj == 0)
                            sp = (jj == SJ - 1)
                            nc.tensor.matmul(apl[0], vt[:, jj, vs],
                                             prevE[:, 0:512], start=st, stop=sp)
                            nc.tensor.matmul(apl[1], vt[:, jj, vs],
                                             prevE[:, 512:1024], start=st, stop=sp)
                        prevE = E if j < SJ else None
                    # move a_plus off PSUM quickly (releases the apl banks)
                    acp = work.tile([65, 2, 512], F32, tag="acp", bufs=3, name="acp")
                    for tg in range(2):
                        nc.vector.tensor_copy(out=acp[:, tg, :], in_=apl[tg])
                    pending_norm = ((h, tsup), acp)
            emit_normalize(*pending_norm)
            emit_proj(NT - 1)

    nc.compile()
    return nc


def get_program():
    global _PROGRAM
    if _PROGRAM is None:
        _PROGRAM = build_program()
    return _PROGRAM


def make_in_maps(x, norm_w, norm_b, qkv_w, qkv_b, proj_w):
    """Build the 8 per-core input maps from full inputs."""
    f = np.float32
    x2 = np.ascontiguousarray(x.reshape(B, C, L), dtype=f)

    gmask = np.zeros((128, 4, G), dtype=f)
    bmask = np.zeros((G, 4, 128), dtype=f)
    for t in range(4):
        for p in range(128):
            g = (t * 128 + p) // 16
            gmask[p, t, g] = 1.0 / 16.0
            bmask[g, t, p] = 1.0
    gamma4 = np.ascontiguousarray(norm_w.reshape(4, 128), dtype=f)
    beta4 = np.ascontiguousarray(norm_b.reshape(4, 128), dtype=f)

    in_maps = []
    for cid in range(N_CORES):
        b = cid // 4
        h0 = 2 * (cid % 4)
        h1 = h0 + 1
        qrows = list(range(192 * h0, 192 * h0 + 64)) + list(range(192 * h1, 192 * h1 + 64))
        krows = [r + 64 for r in qrows]
        v0 = list(range(192 * h0 + 128, 192 * h0 + 192))
        v1 = list(range(192 * h1 + 128, 192 * h1 + 192))
        wqT = np.ascontiguousarray(qkv_w[qrows, :].T, dtype=f)
        wkT = np.ascontiguousarray(qkv_w[krows, :].T, dtype=f)
        wvT = np.zeros((C, 256), dtype=f)
        wvT[:, 0:64] = qkv_w[v0, :].T
        wvT[:, 65:129] = qkv_w[v1, :].T
        qbv = np.ascontiguousarray(qkv_b[qrows], dtype=f)
        kbv = np.ascontiguousarray(qkv_b[krows], dtype=f)
        vbv = np.zeros((130,), dtype=f)
        vbv[0:64] = qkv_b[v0]
        vbv[65:129] = qkv_b[v1]
        vbv[64] = 1.0    # softmax-denominator ones columns (weight cols there are 0)
        vbv[129] = 1.0
        ch_cols = list(range(64 * h0, 64 * h0 + 64)) + list(range(64 * h1, 64 * h1 + 64))
        pwT = np.ascontiguousarray(proj_w[:, ch_cols].T, dtype=f)
        in_maps.append({
            "xb": x2[b], "gmask": gmask, "bmask": bmask,
            "gamma4": gamma4, "beta4": beta4,
            "wqT": wqT, "wkT": wkT, "wvT": wvT,
            "qb": qbv, "kb": kbv, "vb": vbv, "pwT": pwT,
        })
    return in_maps


def kernel(x, norm_w, norm_b, qkv_w, qkv_b, proj_w, proj_b, _trace=False):
    x = np.asarray(x, dtype=np.float32)
    in_maps = make_in_maps(x, np.asarray(norm_w), np.asarray(norm_b),
                           np.asarray(qkv_w), np.asarray(qkv_b), np.asarray(proj_w))
    nc = get_program()
    res = run_bass_kernel_spmd(nc, in_maps, list(range(N_CORES)), trace=_trace)
    hout = np.zeros((B, C, L), dtype=np.float32)
    for cid in range(N_CORES):
        hout[cid // 4] += res.results[cid]["part"]
    hout += np.asarray(proj_b, dtype=np.float32)[None, :, None]
    out = x + hout.reshape(x.shape)
    if _trace:
        return out.astype(np.float32), res
    return out.astype(np.float32)
